# revision 36
# baseline (speedup 1.0000x reference)
"""GCN layer (gather -> weighted scatter-sum -> dense transform) on 8 trn2 cores.

Strategy (1-D row partitioning of destination nodes):
  - Core c owns destination nodes [c*NPW, (c+1)*NPW). edge_dst is sorted, so
    each core's edges are a contiguous slice of the edge list.
  - Within a core, dst nodes are processed in windows of 128 (the PSUM
    partition size). Every window's edges are padded to a fixed number of
    128-edge groups so all 8 cores run the same program.
  - Per 128-edge group:
      * dma_gather pulls the 128 source rows H[src] (fp16) from HBM into an
        SBUF tile G [128 edges x 128 feat] (edge e=j*128+p lands on
        partition p, slot j).
      * DVE builds S [128 edges x 128 nodes] = (iota == dstrel) * w with one
        fused tensor_scalar op.
      * TensorE accumulates aggT[feat, node] += G.T @ S in PSUM.
  - dma_gather indices are int16 (< 32768), so edges are split into a "lo"
    stream (src < 32768, gathered from H[:32768]) and a "hi" stream
    (src >= 32768, gathered from H[32768:]); both accumulate into the same
    PSUM window.
  - Final transform: out.T = W.T @ aggT (+ b) with W stationary, computed in
    512-column chunks; bias is added during the PSUM->SBUF copy (per-partition
    ACT bias, since the output is transposed: partitions = out features).
    The kernel writes out.T [128, NWIN*128] per core; the host transposes and
    concatenates.
"""

import os
import numpy as np

N_CORES = 8
N_NODES = 50000
D = 128
NPW = N_NODES // N_CORES  # 6250 dst nodes per core
WIN = 128
SPLIT = 32768  # int16-addressable row limit for dma_gather

# gather dtype: "f16" (half gather traffic, rel err ~3e-4) or "f32" (exact)
GDTYPE = os.environ.get("GCN_GDTYPE", "f16")
# number of SWDGE queues to round-robin gather calls over (1-4)
NQUEUES = int(os.environ.get("GCN_NQ", "4"))

LAST_EXEC_NS = None  # set when GCN_TRACE=1
LAST_RESULTS = None


def _ceil_div(a, b):
    return -(-a // b)


def _prep(H, edge_src, edge_dst, edge_weight, n_cores=N_CORES):
    """Host-side sharding: per-core, per-window, per-stream edge lists with
    padding to common sizes. Returns per-core arrays + common geometry."""
    nwin = _ceil_div(NPW, WIN)
    # per (core, window, stream) edge index lists
    per_core = []
    max_lo = 0
    max_hi = 0
    max_all = 0
    for c in range(n_cores):
        n0, n1 = c * NPW, (c + 1) * NPW
        e0, e1 = np.searchsorted(edge_dst, [n0, n1])
        d = edge_dst[e0:e1] - n0
        s = edge_src[e0:e1]
        w = edge_weight[e0:e1]
        wins = []
        for wi in range(nwin):
            i0, i1 = np.searchsorted(d, [wi * WIN, wi * WIN + WIN])
            sw, dw, ww = s[i0:i1], d[i0:i1] - wi * WIN, w[i0:i1]
            lo = sw < SPLIT
            wins.append(
                (
                    (sw[lo], dw[lo], ww[lo]),
                    (sw[~lo] - SPLIT, dw[~lo], ww[~lo]),
                    (sw, dw, ww),
                )
            )
            max_lo = max(max_lo, int(lo.sum()))
            max_hi = max(max_hi, int((~lo).sum()))
            max_all = max(max_all, int(i1 - i0))
        per_core.append(wins)
    g_lo = max(1, _ceil_div(max_lo, 128))
    g_hi = max(1, _ceil_div(max_hi, 128))
    g_all = max(1, _ceil_div(max_all, 128))
    return per_core, nwin, g_lo, g_hi, g_all


def _chunks(g, maxg):
    """Split g groups into maxg-sized chunks (last may be smaller). Front-
    loading maxg-size chunks (rather than near-even) minimizes call count per
    window after trimming to the window's effective group count."""
    out = []
    c0 = 0
    while c0 < g:
        k = min(maxg, g - c0)
        out.append((c0, k))
        c0 += k
    return out


def _device_arrays(wins, nwin, g, stream, chunks, np_meta_dtype, regs=None):
    """Build idx (wrapped-16 per gather call; call = (window, chunk)) +
    dstrel/weight arrays for one core and one stream ('lo'=0, 'hi'=1).

    regs is unused (kept for signature compat)."""
    ne = g * 128
    idx = np.zeros((nwin, ne), np.int16)
    drel = np.zeros((nwin, ne), np_meta_dtype)
    wgt = np.zeros((nwin, ne), np_meta_dtype)
    for wi in range(nwin):
        sw, dw, ww = wins[wi][stream]
        n = len(sw)
        idx[wi, :n] = sw.astype(np.int16)
        drel[wi, :n] = dw.astype(np_meta_dtype)
        wgt[wi, :n] = ww.astype(np_meta_dtype)
    parts = []
    for wi in range(nwin):
        for (c0, k) in chunks:
            flat = idx[wi, c0 * 128 : (c0 + k) * 128]
            parts.append(flat.reshape(-1, 16).T)  # [16, k*8]
    idx_dev = np.tile(np.concatenate(parts, axis=1), (8, 1))  # [128, nwin*g*8]
    # meta: [p, w*g + j] = value of edge e=j*128+p in window w
    drel_dev = np.ascontiguousarray(
        drel.reshape(nwin, g, 128).transpose(2, 0, 1).reshape(128, nwin * g)
    )
    wgt_dev = np.ascontiguousarray(
        wgt.reshape(nwin, g, 128).transpose(2, 0, 1).reshape(128, nwin * g)
    )
    return idx_dev, drel_dev, wgt_dev


def _device_arrays_ind(wins, nwin, g, np_meta_dtype):
    """idx (int32, natural [p, w*g+j] layout) + dstrel/weight arrays for the
    combined stream (indirect_dma_start variant)."""
    ne = g * 128
    idx = np.zeros((nwin, ne), np.int32)
    drel = np.zeros((nwin, ne), np_meta_dtype)
    wgt = np.zeros((nwin, ne), np_meta_dtype)
    for wi in range(nwin):
        sw, dw, ww = wins[wi][2]
        n = len(sw)
        idx[wi, :n] = sw
        drel[wi, :n] = dw.astype(np_meta_dtype)
        wgt[wi, :n] = ww.astype(np_meta_dtype)

    def dev(a):
        return np.ascontiguousarray(
            a.reshape(nwin, g, 128).transpose(2, 0, 1).reshape(128, nwin * g)
        )

    return dev(idx), dev(drel), dev(wgt)


def _build_program_ind(nwin, g_all, ch, n_src_rows, n_cores=N_CORES):
    """Indirect_dma_start variant: int32 indices, no lo/hi split."""
    from contextlib import ExitStack

    import concourse.bass as bass
    import concourse.tile as tile
    from concourse import bacc, mybir

    f32 = mybir.dt.float32
    gdt = mybir.dt.float16 if GDTYPE == "f16" else mybir.dt.float32
    i32 = mybir.dt.int32

    nc = bacc.Bacc(
        "TRN2", target_bir_lowering=False, debug=False, num_devices=n_cores,
    )

    npad = nwin * WIN
    h_t = nc.dram_tensor("h_src", [n_src_rows, D], gdt, kind="ExternalInput")
    idx_t = nc.dram_tensor("idx_all", [128, nwin * g_all], i32, kind="ExternalInput")
    drel_t = nc.dram_tensor("drel", [128, nwin * g_all], gdt, kind="ExternalInput")
    wgt_t = nc.dram_tensor("wgt", [128, nwin * g_all], gdt, kind="ExternalInput")
    iota_t = nc.dram_tensor("iota", [128, 128], gdt, kind="ExternalInput")
    w_t = nc.dram_tensor("wmat", [D, D], gdt, kind="ExternalInput")
    b_t = nc.dram_tensor("bcol", [D, 1], f32, kind="ExternalInput")
    out_t = nc.dram_tensor("outT", [D, npad], f32, kind="ExternalOutput")

    with tile.TileContext(nc) as tc:
        with ExitStack() as ctx:
            const = ctx.enter_context(tc.tile_pool(name="const", bufs=1))
            gpool = ctx.enter_context(tc.tile_pool(name="gather", bufs=6))
            spool = ctx.enter_context(tc.tile_pool(name="sel", bufs=3))
            opool = ctx.enter_context(tc.tile_pool(name="outsb", bufs=2))
            ps_agg = ctx.enter_context(tc.tile_pool(name="ps_agg", bufs=2, space="PSUM"))
            ps_out = ctx.enter_context(tc.tile_pool(name="ps_out", bufs=2, space="PSUM"))

            idx = const.tile(list(idx_t.shape), i32)
            drel = const.tile(list(drel_t.shape), gdt)
            wgt = const.tile(list(wgt_t.shape), gdt)
            iota = const.tile([128, 128], gdt)
            wmat = const.tile([D, D], gdt)
            bcol = const.tile([D, 1], f32)
            agg_all = const.tile([128, npad], gdt, tag="agg_all")

            for sb, dr in ((idx, idx_t), (drel, drel_t), (wgt, wgt_t),
                           (iota, iota_t), (wmat, w_t), (bcol, b_t)):
                nc.sync.dma_start(sb[:], dr[:])

            for wg in range(nwin):
                gtiles = []
                for (c0, k) in ch:
                    gt = gpool.tile([128, ch[0][1], 128], gdt, tag="g")
                    nc.gpsimd.indirect_dma_start(
                        out=gt[:, :k, :],
                        out_offset=None,
                        in_=h_t[:],
                        in_offset=bass.IndirectOffsetOnAxis(
                            ap=idx[:, wg * g_all + c0 : wg * g_all + c0 + k],
                            axis=0,
                        ),
                    )
                    gtiles.append((gt, c0, k))

                sh = (128, g_all, 128)
                c0m = wg * g_all
                s = spool.tile([128, g_all, 128], gdt, tag="sel")
                nc.vector.tensor_tensor(
                    s[:], iota[:, None, :].broadcast_to(sh),
                    drel[:, c0m : c0m + g_all, None].broadcast_to(sh),
                    mybir.AluOpType.is_equal,
                )
                nc.vector.tensor_tensor(
                    s[:], s[:], wgt[:, c0m : c0m + g_all, None].broadcast_to(sh),
                    mybir.AluOpType.mult,
                )

                psum = ps_agg.tile([128, 128], f32, tag="psagg")
                k_idx = 0
                for (gt, c0, k) in gtiles:
                    for j in range(k):
                        nc.tensor.matmul(
                            psum[:], gt[:, j, :], s[:, c0 + j, :],
                            start=(k_idx == 0), stop=(k_idx == g_all - 1),
                        )
                        k_idx += 1
                nc.scalar.copy(agg_all[:, wg * WIN : (wg + 1) * WIN], psum[:])

            CH = 512
            for t0 in range(0, npad, CH):
                n = min(CH, npad - t0)
                po = ps_out.tile([128, CH], f32, tag="psout")
                nc.tensor.matmul(
                    po[:, :n], wmat[:], agg_all[:, t0 : t0 + n],
                    start=True, stop=True,
                )
                ob = opool.tile([128, CH], f32, tag="outsb")
                nc.scalar.add(ob[:, :n], po[:, :n], bcol[:])
                nc.sync.dma_start(out_t[:, t0 : t0 + n], ob[:, :n])

    nc.compile()
    return nc


def _build_program(nwin, g_lo, g_hi, ch_lo, ch_hi, n_src_rows, n_cores=N_CORES,
                   gw_lo=None, gw_hi=None):
    """Trace the (single, SPMD-shared) Bass program."""
    from contextlib import ExitStack

    import concourse.bass as bass
    import concourse.tile as tile
    from concourse import bacc, mybir

    f32 = mybir.dt.float32
    gdt = mybir.dt.float16 if GDTYPE == "f16" else mybir.dt.float32
    i16 = mybir.dt.int16

    nc = bacc.Bacc(
        "TRN2",
        target_bir_lowering=False,
        debug=False,
        num_devices=n_cores,
        num_swdge_queues=NQUEUES,
    )
    qctr = [0]

    def next_q():
        q = qctr[0] % NQUEUES
        qctr[0] += 1
        return q

    npad = nwin * WIN
    n_lo_rows = min(SPLIT, n_src_rows)
    n_hi_rows = n_src_rows - n_lo_rows

    h_t = nc.dram_tensor("h_src", [n_src_rows, D], gdt, kind="ExternalInput")
    idx_lo_t = nc.dram_tensor(
        "idx_lo", [128, nwin * g_lo * 8], i16, kind="ExternalInput",
    )
    idx_hi_t = nc.dram_tensor(
        "idx_hi", [128, nwin * g_hi * 8], i16, kind="ExternalInput",
    )
    drel_lo_t = nc.dram_tensor("drel_lo", [128, nwin * g_lo], gdt, kind="ExternalInput")
    wgt_lo_t = nc.dram_tensor("wgt_lo", [128, nwin * g_lo], gdt, kind="ExternalInput")
    drel_hi_t = nc.dram_tensor("drel_hi", [128, nwin * g_hi], gdt, kind="ExternalInput")
    wgt_hi_t = nc.dram_tensor("wgt_hi", [128, nwin * g_hi], gdt, kind="ExternalInput")
    iota_t = nc.dram_tensor("iota", [128, 128], gdt, kind="ExternalInput")
    w_t = nc.dram_tensor("wmat", [D, D], gdt, kind="ExternalInput")
    b_t = nc.dram_tensor("bcol", [D, 1], f32, kind="ExternalInput")
    out_t = nc.dram_tensor("outT", [D, npad], f32, kind="ExternalOutput")

    with tile.TileContext(nc) as tc:
        with ExitStack() as ctx:
            const = ctx.enter_context(tc.tile_pool(name="const", bufs=1))
            gpool = ctx.enter_context(tc.tile_pool(name="gather", bufs=6))
            spool = ctx.enter_context(tc.tile_pool(name="sel", bufs=3))
            opool = ctx.enter_context(tc.tile_pool(name="outsb", bufs=2))
            ps_agg = ctx.enter_context(
                tc.tile_pool(name="ps_agg", bufs=2, space="PSUM")
            )
            ps_out = ctx.enter_context(
                tc.tile_pool(name="ps_out", bufs=2, space="PSUM")
            )

            # resident constants / metadata
            idx_lo = const.tile(list(idx_lo_t.shape), i16)
            idx_hi = const.tile(list(idx_hi_t.shape), i16)
            drel_lo = const.tile(list(drel_lo_t.shape), gdt)
            wgt_lo = const.tile(list(wgt_lo_t.shape), gdt)
            drel_hi = const.tile(list(drel_hi_t.shape), gdt)
            wgt_hi = const.tile(list(wgt_hi_t.shape), gdt)
            iota = const.tile([128, 128], gdt)
            wmat = const.tile([D, D], gdt)
            bcol = const.tile([D, 1], f32)
            agg_all = const.tile([128, npad], gdt, tag="agg_all")

            for sb, dr in (
                (idx_lo, idx_lo_t), (idx_hi, idx_hi_t),
                (drel_lo, drel_lo_t), (wgt_lo, wgt_lo_t),
                (drel_hi, drel_hi_t), (wgt_hi, wgt_hi_t),
                (iota, iota_t), (wmat, w_t), (bcol, b_t),
            ):
                nc.sync.dma_start(sb[:], dr[:])

            h_lo = h_t[0:n_lo_rows, :]
            h_hi = h_t[n_lo_rows:n_src_rows, :] if n_hi_rows > 0 else None
            use_hi = h_hi is not None

            for wg in range(nwin):
                # gather this window's edges: one SWDGE call per chunk.
                # A call of k*128 idxs needs 8k+1 SWDGE ring entries; calls
                # with 97 entries (k=12) crash the exec unit on HW, k<=8 is
                # proven safe.
                # effective groups this window (shared across cores): groups
                # beyond the max valid count are pure padding -> not gathered,
                # not matmul'd. Every issued call is fully valid, so no tile
                # region is ever read without having been written.
                gwl = gw_lo[wg] if gw_lo else g_lo
                gwh = gw_hi[wg] if gw_hi else g_hi
                gtiles_lo = []
                for ci, (c0, k) in enumerate(ch_lo):
                    ke = min(max(gwl - c0, 0), k)
                    if ke == 0:
                        continue
                    gt = gpool.tile([128, k, 128], gdt, tag=f"glo{ci}")
                    col = (wg * g_lo + c0) * 8
                    nc.gpsimd.dma_gather(
                        gt[:, :ke, :], h_lo, idx_lo[:, col : col + ke * 8],
                        num_idxs=ke * 128, num_idxs_reg=ke * 128, elem_size=D,
                        queue_num=next_q(),
                    )
                    gtiles_lo.append((gt, c0, ke))
                gtiles_hi = []
                if use_hi:
                    for ci, (c0, k) in enumerate(ch_hi):
                        ke = min(max(gwh - c0, 0), k)
                        if ke == 0:
                            continue
                        gt = gpool.tile([128, k, 128], gdt, tag=f"ghi{ci}")
                        col = (wg * g_hi + c0) * 8
                        nc.gpsimd.dma_gather(
                            gt[:, :ke, :], h_hi, idx_hi[:, col : col + ke * 8],
                            num_idxs=ke * 128, num_idxs_reg=ke * 128, elem_size=D,
                            queue_num=next_q(),
                        )
                        gtiles_hi.append((gt, c0, ke))

                # S for the window's effective groups in 2 DVE ops per stream:
                # S[p, j, n] = (n == drel[p, j]) * w[p, j] via step-0
                # broadcast APs on both operands.
                def build_s(meta_d, meta_w, g, gw, tag):
                    s = spool.tile([128, g, 128], gdt, tag=tag)
                    sh = (128, gw, 128)
                    c0m = wg * g
                    nc.vector.tensor_tensor(
                        s[:, :gw, :], iota[:, None, :].broadcast_to(sh),
                        meta_d[:, c0m : c0m + gw, None].broadcast_to(sh),
                        mybir.AluOpType.is_equal,
                    )
                    nc.vector.tensor_tensor(
                        s[:, :gw, :], s[:, :gw, :],
                        meta_w[:, c0m : c0m + gw, None].broadcast_to(sh),
                        mybir.AluOpType.mult,
                    )
                    return s

                s_lo = build_s(drel_lo, wgt_lo, g_lo, gwl, "slo")
                s_hi = build_s(drel_hi, wgt_hi, g_hi, gwh, "shi") if use_hi else None

                psum = ps_agg.tile([128, 128], f32, tag="psagg")
                n_groups = sum(k for _, _, k in gtiles_lo)
                n_groups += sum(k for _, _, k in gtiles_hi)
                k_idx = 0
                for (gt, c0, k), s_all in (
                    [(t, s_lo) for t in gtiles_lo]
                    + [(t, s_hi) for t in gtiles_hi]
                ):
                    for j in range(k):
                        nc.tensor.matmul(
                            psum[:], gt[:, j, :], s_all[:, c0 + j, :],
                            start=(k_idx == 0), stop=(k_idx == n_groups - 1),
                        )
                        k_idx += 1
                # aggT window -> SBUF (cast to gather dtype)
                nc.scalar.copy(agg_all[:, wg * WIN : (wg + 1) * WIN], psum[:])

            # out.T = W.T @ aggT + b, in 512-column chunks
            CH = 512
            for t0 in range(0, npad, CH):
                n = min(CH, npad - t0)
                po = ps_out.tile([128, CH], f32, tag="psout")
                nc.tensor.matmul(
                    po[:, :n], wmat[:], agg_all[:, t0 : t0 + n],
                    start=True, stop=True,
                )
                ob = opool.tile([128, CH], f32, tag="outsb")
                nc.scalar.add(ob[:, :n], po[:, :n], bcol[:])
                nc.sync.dma_start(out_t[:, t0 : t0 + n], ob[:, :n])

    nc.compile()
    return nc


def _prep_merged(H, edge_src, edge_dst, edge_weight, n_cores=N_CORES):
    """One gather stream per core: per dst-window, unique srcs sorted
    ascending (even windows) / descending (odd windows) so that consecutive
    windows' slot values are continuous and any 8-group call spans a small
    src range (addressable with int16 idx off a per-call base row)."""
    nwin = _ceil_div(NPW, WIN)
    per_core = []
    for c in range(n_cores):
        n0, n1 = c * NPW, (c + 1) * NPW
        e0, e1 = np.searchsorted(edge_dst, [n0, n1])
        d = edge_dst[e0:e1] - n0
        s = edge_src[e0:e1]
        w = edge_weight[e0:e1]
        wins = []
        for wi in range(nwin):
            i0, i1 = np.searchsorted(d, [wi * WIN, wi * WIN + WIN])
            sw, dw, ww = s[i0:i1], d[i0:i1] - wi * WIN, w[i0:i1]
            if len(sw):
                u, inv = np.unique(sw, return_inverse=True)
                if wi % 2 == 1:
                    u = u[::-1].copy()
                    inv = len(u) - 1 - inv
            else:
                u, inv = sw, np.zeros(0, np.int64)
            wins.append((u, dw, ww, inv))
        per_core.append(wins)
    return per_core, nwin


def _merged_geom(per_core, nwin, maxg):
    """Group counts/offsets + per-call base rows (shared across cores)."""
    gw = []
    for wi in range(nwin):
        m = max(len(wins[wi][0]) for wins in per_core)
        gw.append(max(1, _ceil_div(m, 128)))
    off = [0]
    for wi in range(nwin):
        off.append(off[-1] + gw[wi])
    G = off[-1]
    ncall = _ceil_div(G, maxg)
    # per-call min/max real src value across cores
    bases = []
    for c in range(ncall):
        g0, g1 = c * maxg, min((c + 1) * maxg, G)
        lo_v, hi_v = N_NODES, -1
        for wins in per_core:
            for wi in range(nwin):
                if off[wi + 1] <= g0 or off[wi] >= g1:
                    continue
                u = wins[wi][0]
                if not len(u):
                    continue
                a = max(0, (g0 - off[wi]) * 128)
                b = min(len(u), (g1 - off[wi]) * 128)
                if a < b:
                    seg = u[a:b]
                    lo_v = min(lo_v, int(seg.min()))
                    hi_v = max(hi_v, int(seg.max()))
        assert hi_v >= 0, f"call {c} has no real edges"
        assert hi_v - lo_v < 32768, (c, lo_v, hi_v)
        bases.append(lo_v)
    return gw, off, G, ncall, bases


def _merged_arrays(wins, nwin, gw, off, G, ncall, bases, maxg, np_g):
    """Per-core merged idx stream (per-call base-relative, wrapped-16) and
    w-baked S stream."""
    idx = np.zeros(ncall * maxg * 128, np.int32)
    for wi in range(nwin):
        u = wins[wi][0]
        idx[off[wi] * 128 : off[wi] * 128 + len(u)] = u
    # pad slots hold 0 (absolute); make them base-relative 0 instead
    parts = []
    for c in range(ncall):
        fl = idx[c * maxg * 128 : (c + 1) * maxg * 128] - bases[c]
        real = np.zeros(maxg * 128, bool)
        g0, g1 = c * maxg, min((c + 1) * maxg, G)
        for wi in range(nwin):
            a = max(off[wi], g0)
            b = min(off[wi + 1], g1)
            if a < b:
                u_n = len(wins[wi][0])
                s0 = (a - off[wi]) * 128
                s1 = min(u_n, (b - off[wi]) * 128)
                if s1 > s0:
                    lo = a * 128 - c * maxg * 128
                    real[lo : lo + (s1 - s0)] = True
        fl[~real] = 0
        assert fl.min() >= 0 and fl.max() < 32768, (c, fl.min(), fl.max())
        parts.append(fl.astype(np.int16).reshape(-1, 16).T)
    idx_dev = np.tile(np.concatenate(parts, axis=1), (8, 1))

    s_arr = np.zeros((128, G * 128), np.float32)
    for wi in range(nwin):
        _, dw, ww, inv = wins[wi]
        if len(dw) == 0:
            continue
        np.add.at(
            s_arr,
            (inv % 128, (off[wi] + inv // 128) * 128 + dw),
            ww.astype(np.float32),
        )
    return idx_dev, s_arr.astype(np_g)


def _straddle_geom(per_core, nwin, maxg):
    """Slot-space geometry with NO per-window group padding: window w owns
    flat slots [M[w], M[w]+m[w]); groups of 128 slots may straddle window
    boundaries (such a group feeds both windows' matmuls with different,
    host-zeroed S blocks)."""
    m = [max(len(wins[wi][0]) for wins in per_core) for wi in range(nwin)]
    M = [0]
    for wi in range(nwin):
        M.append(M[-1] + m[wi])
    G = _ceil_div(M[-1], 128)
    ncall = _ceil_div(G, maxg)
    wg0 = [M[wi] // 128 for wi in range(nwin)]
    wg1 = [_ceil_div(M[wi + 1], 128) for wi in range(nwin)]
    soff = [0]
    for wi in range(nwin):
        soff.append(soff[-1] + wg1[wi] - wg0[wi])
    # per-call base rows (shared across cores)
    bases = []
    for c in range(ncall):
        s0, s1 = c * maxg * 128, min((c + 1) * maxg * 128, G * 128)
        lo_v, hi_v = N_NODES, -1
        for wins in per_core:
            for wi in range(nwin):
                if M[wi + 1] <= s0 or M[wi] >= s1:
                    continue
                u = wins[wi][0]
                a = max(0, s0 - M[wi])
                b = min(len(u), s1 - M[wi])
                if a < b:
                    seg = u[a:b]
                    lo_v = min(lo_v, int(seg.min()))
                    hi_v = max(hi_v, int(seg.max()))
        assert hi_v >= 0, f"call {c} has no real edges"
        assert hi_v - lo_v < 32768, (c, lo_v, hi_v)
        bases.append(lo_v)
    return m, M, G, ncall, wg0, wg1, soff, bases


def _straddle_arrays(wins, nwin, m, M, G, ncall, wg0, wg1, soff, bases, maxg,
                     np_g):
    """Per-core idx stream (per-call base-relative) + S stream where each
    window's blocks cover its overlapping groups (zero outside the window)."""
    idx = np.zeros(ncall * maxg * 128, np.int32)
    for wi in range(nwin):
        u = wins[wi][0]
        idx[M[wi] : M[wi] + len(u)] = u
    parts = []
    for c in range(ncall):
        s0, s1 = c * maxg * 128, (c + 1) * maxg * 128
        fl = idx[s0:s1] - bases[c]
        real = np.zeros(maxg * 128, bool)
        for wi in range(nwin):
            if M[wi + 1] <= s0 or M[wi] >= s1:
                continue
            a = max(M[wi], s0)
            b = min(M[wi] + len(wins[wi][0]), s1)
            if a < b:
                real[a - s0 : b - s0] = True
        fl[~real] = 0
        assert fl.min() >= 0 and fl.max() < 32768, (c, fl.min(), fl.max())
        parts.append(fl.astype(np.int16).reshape(-1, 16).T)
    idx_dev = np.tile(np.concatenate(parts, axis=1), (8, 1))

    s_arr = np.zeros((128, soff[-1] * 128), np.float32)
    for wi in range(nwin):
        _, dw, ww, inv = wins[wi]
        if len(dw) == 0:
            continue
        slot = M[wi] + inv
        blk = slot // 128 - wg0[wi]
        np.add.at(
            s_arr,
            (slot % 128, (soff[wi] + blk) * 128 + dw),
            ww.astype(np.float32),
        )
    return idx_dev, s_arr.astype(np_g)


def _dedup_wins(per_core, nwin):
    """Per (core, window, stream): unique sources + per-edge slot mapping.
    Edges sharing a src within a window share one gather slot; their weights
    land in different (or the same, summed) columns of that slot's S row."""
    out = []
    for wins in per_core:
        w2 = []
        for wi in range(nwin):
            entry = []
            for stream in (0, 1):
                sw, dw, ww = wins[wi][stream]
                if len(sw):
                    u, inv = np.unique(sw, return_inverse=True)
                else:
                    u, inv = sw, np.zeros(0, np.int64)
                entry.append((u, dw, ww, inv))
            w2.append(entry)
        out.append(w2)
    return out


def _flat_geom(per_core, nwin, maxg):
    """Flat-stream geometry: per-window effective (trimmed) group counts,
    flat-stream group offsets, call counts, and S-stream group offsets."""
    gw_lo, gw_hi = [], []
    for wi in range(nwin):
        m_lo = max(len(wins[wi][0][0]) for wins in per_core)
        m_hi = max(len(wins[wi][1][0]) for wins in per_core)
        gw_lo.append(max(1, _ceil_div(m_lo, 128)))
        gw_hi.append(max(1, _ceil_div(m_hi, 128)))
    off_lo = [0]
    off_hi = [0]
    soff = [0]
    for wi in range(nwin):
        off_lo.append(off_lo[-1] + gw_lo[wi])
        off_hi.append(off_hi[-1] + gw_hi[wi])
        soff.append(soff[-1] + gw_lo[wi] + gw_hi[wi])
    G_lo, G_hi = off_lo[-1], off_hi[-1]
    ncall_lo = _ceil_div(G_lo, maxg)
    ncall_hi = _ceil_div(G_hi, maxg)
    return gw_lo, gw_hi, off_lo, off_hi, G_lo, G_hi, ncall_lo, ncall_hi, soff


def _flat_arrays(wins, nwin, gw_lo, gw_hi, off_lo, off_hi, ncall_lo, ncall_hi,
                 soff, maxg, np_g):
    """Per-core flat idx streams (wrapped-16 per call) + w-baked S stream."""

    def idx_stream(stream_id, gw, off, ncall):
        idx = np.zeros(ncall * maxg * 128, np.int16)
        for wi in range(nwin):
            sw = wins[wi][stream_id][0]
            idx[off[wi] * 128 : off[wi] * 128 + len(sw)] = sw.astype(np.int16)
        parts = [
            idx[c * maxg * 128 : (c + 1) * maxg * 128].reshape(-1, 16).T
            for c in range(ncall)
        ]
        return np.tile(np.concatenate(parts, axis=1), (8, 1))

    idx_lo = idx_stream(0, gw_lo, off_lo, ncall_lo)
    idx_hi = idx_stream(1, gw_hi, off_hi, ncall_hi)

    # S with w baked in; edges sharing a gather slot (dedup) accumulate into
    # that slot's row (fp32 accumulate, cast once at the end).
    s_arr = np.zeros((128, soff[-1] * 128), np.float32)
    for wi in range(nwin):
        for stream_id, gbase in ((0, soff[wi]), (1, soff[wi] + gw_lo[wi])):
            _, dw, ww, inv = wins[wi][stream_id]
            if len(dw) == 0:
                continue
            slot = inv
            np.add.at(
                s_arr,
                (slot % 128, (gbase + slot // 128) * 128 + dw),
                ww.astype(np.float32),
            )
    return idx_lo, idx_hi, s_arr.astype(np_g)


def _build_program_v3(nwin, gw, off, G, ncall, bases, maxg, n_src_rows,
                      n_cores=N_CORES):
    """Merged single-table stream: per-call base row (compile-time), flat
    8-group calls, host-streamed S, interleaved final transform."""
    from contextlib import ExitStack

    import concourse.tile as tile
    from concourse import bacc, mybir

    f32 = mybir.dt.float32
    gdt = mybir.dt.float16 if GDTYPE == "f16" else mybir.dt.float32
    i16 = mybir.dt.int16

    nc = bacc.Bacc(
        "TRN2", target_bir_lowering=False, debug=False, num_devices=n_cores,
        num_swdge_queues=NQUEUES,
    )
    qctr = [0]

    def next_q():
        q = qctr[0] % NQUEUES
        qctr[0] += 1
        return q

    npad = nwin * WIN
    gmax = max(gw)

    h_t = nc.dram_tensor("h_src", [n_src_rows, D], gdt, kind="ExternalInput")
    idx_t = nc.dram_tensor("idx", [128, ncall * maxg * 8], i16, kind="ExternalInput")
    s_t = nc.dram_tensor("s_sel", [128, G * 128], gdt, kind="ExternalInput")
    w_t = nc.dram_tensor("wmat", [D, D], gdt, kind="ExternalInput")
    b_t = nc.dram_tensor("bcol", [D, 1], f32, kind="ExternalInput")
    out_t = nc.dram_tensor("outT", [D, npad], f32, kind="ExternalOutput")

    with tile.TileContext(nc) as tc:
        with ExitStack() as ctx:
            const = ctx.enter_context(tc.tile_pool(name="const", bufs=1))
            gpool = ctx.enter_context(tc.tile_pool(name="gather", bufs=14))
            spool = ctx.enter_context(tc.tile_pool(name="sel", bufs=4))
            opool = ctx.enter_context(tc.tile_pool(name="outsb", bufs=2))
            ps_agg = ctx.enter_context(tc.tile_pool(name="ps_agg", bufs=2, space="PSUM"))
            ps_out = ctx.enter_context(tc.tile_pool(name="ps_out", bufs=2, space="PSUM"))

            idx = const.tile(list(idx_t.shape), i16)
            wmat = const.tile([D, D], gdt)
            bcol = const.tile([D, 1], f32)
            agg_all = const.tile([128, npad], gdt, tag="agg_all")

            nc.sync.dma_start(wmat[:], w_t[:])
            nc.sync.dma_start(bcol[:], b_t[:])
            ncol = idx_t.shape[1]
            step = _ceil_div(ncol, 6)
            for a in range(0, ncol, step):
                bnd = min(a + step, ncol)
                nc.sync.dma_start(idx[:, a:bnd], idx_t[:, a:bnd])

            tiles = {}
            issued = [0]
            nreg = {}  # ke -> hoisted num_idxs register (kills per-call MOVE)

            def nref(ke):
                if ke not in nreg:
                    nreg[ke] = nc.gpsimd.to_reg(ke * 128)
                return nreg[ke]

            def issue_through(need_groups):
                while issued[0] * maxg < need_groups:
                    c = issued[0]
                    ke = min(maxg, G - c * maxg)
                    base = bases[c]
                    span = min(32768, n_src_rows - base)
                    gt = gpool.tile([128, maxg, 128], gdt, tag="g")
                    nc.gpsimd.dma_gather(
                        gt[:, :ke, :], h_t[base : base + span, :],
                        idx[:, c * maxg * 8 : c * maxg * 8 + ke * 8],
                        num_idxs=ke * 128, num_idxs_reg=nref(ke),
                        elem_size=D, queue_num=next_q(),
                    )
                    tiles[c] = gt
                    issued[0] += 1

            for wg in range(nwin):
                issue_through(off[wg] + gw[wg])

                gtot = gw[wg]
                s2 = spool.tile([128, gmax * 128], gdt, tag="s")
                nc.sync.dma_start(
                    s2[:, : gtot * 128],
                    s_t[:, off[wg] * 128 : (off[wg] + gtot) * 128],
                )

                psum = ps_agg.tile([128, 128], f32, tag="psagg")
                for j in range(gtot):
                    fg = off[wg] + j
                    nc.tensor.matmul(
                        psum[:], tiles[fg // maxg][:, fg % maxg, :],
                        s2[:, j * 128 : (j + 1) * 128],
                        start=(j == 0), stop=(j == gtot - 1),
                    )
                nc.scalar.copy(agg_all[:, wg * WIN : (wg + 1) * WIN], psum[:])

                CH = 512
                done = (wg + 1) * WIN
                emit = []
                if done % CH == 0:
                    emit.append(done - CH)
                if wg == nwin - 1 and npad % CH != 0:
                    emit.append(npad - npad % CH)
                for t0 in emit:
                    n = min(CH, npad - t0)
                    po = ps_out.tile([128, CH], f32, tag="psout")
                    nc.tensor.matmul(
                        po[:, :n], wmat[:], agg_all[:, t0 : t0 + n],
                        start=True, stop=True,
                    )
                    ob = opool.tile([128, CH], f32, tag="outsb")
                    nc.scalar.add(ob[:, :n], po[:, :n], bcol[:])
                    # out DMA on the activation engine's HWDGE queue so it
                    # never delays the sync queue's S-stream loads
                    nc.scalar.dma_start(out_t[:, t0 : t0 + n], ob[:, :n])

    nc.compile()
    return nc


def _build_program_v4(nwin, m, M, G, ncall, wg0, wg1, soff, bases, maxg,
                      n_src_rows, n_cores=N_CORES):
    """Straddle variant: zero group padding; a boundary-straddling group
    feeds both adjacent windows' matmuls with different S blocks."""
    from contextlib import ExitStack

    import concourse.tile as tile
    from concourse import bacc, mybir

    f32 = mybir.dt.float32
    gdt = mybir.dt.float16 if GDTYPE == "f16" else mybir.dt.float32
    i16 = mybir.dt.int16

    nc = bacc.Bacc(
        "TRN2", target_bir_lowering=False, debug=False, num_devices=n_cores,
        num_swdge_queues=NQUEUES,
    )
    qctr = [0]

    def next_q():
        q = qctr[0] % NQUEUES
        qctr[0] += 1
        return q

    npad = nwin * WIN
    gmax = max(wg1[i] - wg0[i] for i in range(nwin))

    h_t = nc.dram_tensor("h_src", [n_src_rows, D], gdt, kind="ExternalInput")
    idx_t = nc.dram_tensor("idx", [128, ncall * maxg * 8], i16, kind="ExternalInput")
    s_t = nc.dram_tensor("s_sel", [128, soff[-1] * 128], gdt, kind="ExternalInput")
    w_t = nc.dram_tensor("wmat", [D, D], gdt, kind="ExternalInput")
    b_t = nc.dram_tensor("bcol", [D, 1], f32, kind="ExternalInput")
    out_t = nc.dram_tensor("outT", [D, npad], f32, kind="ExternalOutput")

    with tile.TileContext(nc) as tc:
        with ExitStack() as ctx:
            const = ctx.enter_context(tc.tile_pool(name="const", bufs=1))
            gpool = ctx.enter_context(tc.tile_pool(name="gather", bufs=20))
            spool = ctx.enter_context(tc.tile_pool(name="sel", bufs=5))
            opool = ctx.enter_context(tc.tile_pool(name="outsb", bufs=2))
            ps_agg = ctx.enter_context(tc.tile_pool(name="ps_agg", bufs=2, space="PSUM"))
            ps_out = ctx.enter_context(tc.tile_pool(name="ps_out", bufs=2, space="PSUM"))

            idx = const.tile(list(idx_t.shape), i16)
            wmat = const.tile([D, D], gdt)
            bcol = const.tile([D, 1], f32)
            agg_all = const.tile([128, npad], gdt, tag="agg_all")

            nc.sync.dma_start(wmat[:], w_t[:])
            nc.sync.dma_start(bcol[:], b_t[:])
            ncol = idx_t.shape[1]
            step = _ceil_div(ncol, 12)
            for a in range(0, ncol, step):
                bnd = min(a + step, ncol)
                nc.sync.dma_start(idx[:, a:bnd], idx_t[:, a:bnd])

            tiles = {}
            issued = [0]
            nreg = {}

            def nref(ke):
                if ke not in nreg:
                    nreg[ke] = nc.gpsimd.to_reg(ke * 128)
                return nreg[ke]

            def issue_through(need_groups):
                while issued[0] * maxg < need_groups:
                    c = issued[0]
                    ke = min(maxg, G - c * maxg)
                    base = bases[c]
                    span = min(32768, n_src_rows - base)
                    gt = gpool.tile([128, maxg, 128], gdt, tag="g")
                    nc.gpsimd.dma_gather(
                        gt[:, :ke, :], h_t[base : base + span, :],
                        idx[:, c * maxg * 8 : c * maxg * 8 + ke * 8],
                        num_idxs=ke * 128, num_idxs_reg=nref(ke),
                        elem_size=D, queue_num=next_q(),
                    )
                    tiles[c] = gt
                    issued[0] += 1

            for wg in range(nwin):
                issue_through(wg1[wg])

                nblk = wg1[wg] - wg0[wg]
                s2 = spool.tile([128, gmax * 128], gdt, tag="s")
                nc.sync.dma_start(
                    s2[:, : nblk * 128],
                    s_t[:, soff[wg] * 128 : (soff[wg] + nblk) * 128],
                )

                psum = ps_agg.tile([128, 128], f32, tag="psagg")
                for bi in range(nblk):
                    g = wg0[wg] + bi
                    nc.tensor.matmul(
                        psum[:], tiles[g // maxg][:, g % maxg, :],
                        s2[:, bi * 128 : (bi + 1) * 128],
                        start=(bi == 0), stop=(bi == nblk - 1),
                    )
                nc.scalar.copy(agg_all[:, wg * WIN : (wg + 1) * WIN], psum[:])

                CH = 512
                done = (wg + 1) * WIN
                emit = []
                if done % CH == 0:
                    emit.append(done - CH)
                if wg == nwin - 1 and npad % CH != 0:
                    emit.append(npad - npad % CH)
                for t0 in emit:
                    n = min(CH, npad - t0)
                    po = ps_out.tile([128, CH], f32, tag="psout")
                    nc.tensor.matmul(
                        po[:, :n], wmat[:], agg_all[:, t0 : t0 + n],
                        start=True, stop=True,
                    )
                    ob = opool.tile([128, CH], f32, tag="outsb")
                    nc.scalar.add(ob[:, :n], po[:, :n], bcol[:])
                    nc.scalar.dma_start(out_t[:, t0 : t0 + n], ob[:, :n])

    nc.compile()
    return nc


def _build_program_v2(nwin, gw_lo, gw_hi, off_lo, off_hi, G_lo, G_hi,
                      ncall_lo, ncall_hi, soff, maxg, n_src_rows,
                      n_cores=N_CORES):
    """Flat gather streams (calls span window boundaries -> minimal call
    count) + host-precomputed S stream (w baked in) -> no DVE work."""
    from contextlib import ExitStack

    import concourse.tile as tile
    from concourse import bacc, mybir

    f32 = mybir.dt.float32
    gdt = mybir.dt.float16 if GDTYPE == "f16" else mybir.dt.float32
    i16 = mybir.dt.int16

    nc = bacc.Bacc(
        "TRN2", target_bir_lowering=False, debug=False, num_devices=n_cores,
        num_swdge_queues=NQUEUES,
    )
    qctr = [0]

    def next_q():
        q = qctr[0] % NQUEUES
        qctr[0] += 1
        return q

    npad = nwin * WIN
    gmax = max(gw_lo[i] + gw_hi[i] for i in range(nwin))
    n_lo_rows = min(SPLIT, n_src_rows)

    h_t = nc.dram_tensor("h_src", [n_src_rows, D], gdt, kind="ExternalInput")
    idx_lo_t = nc.dram_tensor(
        "idx_lo", [128, ncall_lo * maxg * 8], i16, kind="ExternalInput")
    idx_hi_t = nc.dram_tensor(
        "idx_hi", [128, ncall_hi * maxg * 8], i16, kind="ExternalInput")
    s_t = nc.dram_tensor("s_sel", [128, soff[-1] * 128], gdt, kind="ExternalInput")
    w_t = nc.dram_tensor("wmat", [D, D], gdt, kind="ExternalInput")
    b_t = nc.dram_tensor("bcol", [D, 1], f32, kind="ExternalInput")
    out_t = nc.dram_tensor("outT", [D, npad], f32, kind="ExternalOutput")

    with tile.TileContext(nc) as tc:
        with ExitStack() as ctx:
            const = ctx.enter_context(tc.tile_pool(name="const", bufs=1))
            gpool = ctx.enter_context(tc.tile_pool(name="gather", bufs=10))
            spool = ctx.enter_context(tc.tile_pool(name="sel", bufs=3))
            opool = ctx.enter_context(tc.tile_pool(name="outsb", bufs=2))
            ps_agg = ctx.enter_context(tc.tile_pool(name="ps_agg", bufs=2, space="PSUM"))
            ps_out = ctx.enter_context(tc.tile_pool(name="ps_out", bufs=2, space="PSUM"))

            idx_lo = const.tile(list(idx_lo_t.shape), i16)
            idx_hi = const.tile(list(idx_hi_t.shape), i16)
            wmat = const.tile([D, D], gdt)
            bcol = const.tile([D, 1], f32)
            agg_all = const.tile([128, npad], gdt, tag="agg_all")

            # load idx streams in column chunks so the first gather only
            # depends on the first chunk, not the whole stream
            nc.sync.dma_start(wmat[:], w_t[:])
            nc.sync.dma_start(bcol[:], b_t[:])
            # interleave lo/hi chunks so window 0's hi gathers aren't stuck
            # behind the whole lo stream
            def chunk_ranges(dr, nch=4):
                ncol = dr.shape[1]
                step = _ceil_div(ncol, nch)
                return [(a, min(a + step, ncol)) for a in range(0, ncol, step)]

            for (la, lb), (ha, hb) in zip(chunk_ranges(idx_lo_t),
                                          chunk_ranges(idx_hi_t)):
                nc.sync.dma_start(idx_lo[:, la:lb], idx_lo_t[:, la:lb])
                nc.sync.dma_start(idx_hi[:, ha:hb], idx_hi_t[:, ha:hb])

            h_lo = h_t[0:n_lo_rows, :]
            h_hi = h_t[n_lo_rows:n_src_rows, :]

            lo_tiles = {}
            hi_tiles = {}
            issued = [0, 0]

            def issue_through(which, need_groups):
                G, ncall, tiles, h_ap, idx, tag = (
                    (G_lo, ncall_lo, lo_tiles, h_lo, idx_lo, "glo")
                    if which == 0
                    else (G_hi, ncall_hi, hi_tiles, h_hi, idx_hi, "ghi")
                )
                while issued[which] * maxg < need_groups:
                    c = issued[which]
                    ke = min(maxg, G - c * maxg)
                    gt = gpool.tile([128, maxg, 128], gdt, tag=tag)
                    nc.gpsimd.dma_gather(
                        gt[:, :ke, :], h_ap,
                        idx[:, c * maxg * 8 : c * maxg * 8 + ke * 8],
                        num_idxs=ke * 128, num_idxs_reg=ke * 128,
                        elem_size=D, queue_num=next_q(),
                    )
                    tiles[c] = gt
                    issued[which] += 1

            for wg in range(nwin):
                issue_through(0, off_lo[wg] + gw_lo[wg])
                issue_through(1, off_hi[wg] + gw_hi[wg])

                gtot = gw_lo[wg] + gw_hi[wg]
                s2 = spool.tile([128, gmax * 128], gdt, tag="s")
                nc.sync.dma_start(
                    s2[:, : gtot * 128],
                    s_t[:, soff[wg] * 128 : (soff[wg] + gtot) * 128],
                )

                psum = ps_agg.tile([128, 128], f32, tag="psagg")
                k_idx = 0
                for j in range(gw_lo[wg]):
                    fg = off_lo[wg] + j
                    nc.tensor.matmul(
                        psum[:], lo_tiles[fg // maxg][:, fg % maxg, :],
                        s2[:, j * 128 : (j + 1) * 128],
                        start=(k_idx == 0), stop=(k_idx == gtot - 1),
                    )
                    k_idx += 1
                for j in range(gw_hi[wg]):
                    fg = off_hi[wg] + j
                    col = (gw_lo[wg] + j) * 128
                    nc.tensor.matmul(
                        psum[:], hi_tiles[fg // maxg][:, fg % maxg, :],
                        s2[:, col : col + 128],
                        start=(k_idx == 0), stop=(k_idx == gtot - 1),
                    )
                    k_idx += 1
                nc.scalar.copy(agg_all[:, wg * WIN : (wg + 1) * WIN], psum[:])

            CH = 512
            for t0 in range(0, npad, CH):
                n = min(CH, npad - t0)
                po = ps_out.tile([128, CH], f32, tag="psout")
                nc.tensor.matmul(
                    po[:, :n], wmat[:], agg_all[:, t0 : t0 + n],
                    start=True, stop=True,
                )
                ob = opool.tile([128, CH], f32, tag="outsb")
                nc.scalar.add(ob[:, :n], po[:, :n], bcol[:])
                nc.sync.dma_start(out_t[:, t0 : t0 + n], ob[:, :n])

    nc.compile()
    return nc


def _make_in_maps(H, edge_src, edge_dst, edge_weight, W, b, per_core, nwin,
                  g_lo, g_hi, ch_lo, ch_hi):
    np_g = np.float16 if GDTYPE == "f16" else np.float32
    h_src = np.ascontiguousarray(H.astype(np_g))
    iota = np.tile(np.arange(128, dtype=np_g), (128, 1))
    wmat = np.ascontiguousarray(W.astype(np_g))
    bcol = np.ascontiguousarray(b.astype(np.float32).reshape(D, 1))
    in_maps = []
    for wins in per_core:
        idx_lo, drel_lo, wgt_lo = _device_arrays(wins, nwin, g_lo, 0, ch_lo, np_g)
        idx_hi, drel_hi, wgt_hi = _device_arrays(wins, nwin, g_hi, 1, ch_hi, np_g)
        in_maps.append(
            {
                "h_src": h_src,
                "idx_lo": idx_lo, "idx_hi": idx_hi,
                "drel_lo": drel_lo, "wgt_lo": wgt_lo,
                "drel_hi": drel_hi, "wgt_hi": wgt_hi,
                "iota": iota, "wmat": wmat, "bcol": bcol,
            }
        )
    return in_maps


def kernel(H, edge_src, edge_dst, edge_weight, W, b):
    global LAST_EXEC_NS
    from concourse import bass_utils

    H = np.asarray(H, dtype=np.float32)
    edge_src = np.asarray(edge_src, dtype=np.int32)
    edge_dst = np.asarray(edge_dst, dtype=np.int32)
    edge_weight = np.asarray(edge_weight, dtype=np.float32)
    W = np.asarray(W, dtype=np.float32)
    b = np.asarray(b, dtype=np.float32)

    per_core, nwin, g_lo, g_hi, g_all = _prep(H, edge_src, edge_dst, edge_weight)
    mode = os.environ.get("GCN_GATHER", "straddle")
    if mode == "indirect":
        maxg = int(os.environ.get("GCN_MAXG", "8"))
        ch = _chunks(g_all, maxg)
        nc = _build_program_ind(nwin, g_all, ch, N_NODES)
        np_g = np.float16 if GDTYPE == "f16" else np.float32
        h_src = np.ascontiguousarray(H.astype(np_g))
        iota = np.tile(np.arange(128, dtype=np_g), (128, 1))
        wmat = np.ascontiguousarray(W.astype(np_g))
        bcol = np.ascontiguousarray(b.astype(np.float32).reshape(D, 1))
        in_maps = []
        for wins in per_core:
            idx_all, drel, wgt = _device_arrays_ind(wins, nwin, g_all, np_g)
            in_maps.append({
                "h_src": h_src, "idx_all": idx_all, "drel": drel, "wgt": wgt,
                "iota": iota, "wmat": wmat, "bcol": bcol,
            })
    elif mode == "straddle":
        # merged stream with zero group padding (groups straddle windows)
        maxg = int(os.environ.get("GCN_MAXG", "8"))
        per_core_m, nwin_m = _prep_merged(H, edge_src, edge_dst, edge_weight)
        geom = _straddle_geom(per_core_m, nwin_m, maxg)
        m_, M_, G, ncall, wg0, wg1, soff, bases = geom
        nc = _build_program_v4(nwin_m, *geom, maxg, N_NODES)
        np_g = np.float16 if GDTYPE == "f16" else np.float32
        h_src = np.ascontiguousarray(H.astype(np_g))
        wmat = np.ascontiguousarray(W.astype(np_g))
        bcol = np.ascontiguousarray(b.astype(np.float32).reshape(D, 1))
        in_maps = []
        for wins in per_core_m:
            idx_dev, s_arr = _straddle_arrays(
                wins, nwin_m, m_, M_, G, ncall, wg0, wg1, soff, bases, maxg,
                np_g)
            in_maps.append({
                "h_src": h_src, "idx": idx_dev, "s_sel": s_arr,
                "wmat": wmat, "bcol": bcol,
            })
    elif mode == "merged":
        # single-table boustrophedon stream, per-call base rows
        maxg = int(os.environ.get("GCN_MAXG", "8"))
        per_core_m, nwin_m = _prep_merged(H, edge_src, edge_dst, edge_weight)
        gw, off, G, ncall, bases = _merged_geom(per_core_m, nwin_m, maxg)
        nc = _build_program_v3(nwin_m, gw, off, G, ncall, bases, maxg, N_NODES)
        np_g = np.float16 if GDTYPE == "f16" else np.float32
        h_src = np.ascontiguousarray(H.astype(np_g))
        wmat = np.ascontiguousarray(W.astype(np_g))
        bcol = np.ascontiguousarray(b.astype(np.float32).reshape(D, 1))
        in_maps = []
        for wins in per_core_m:
            idx_dev, s_arr = _merged_arrays(
                wins, nwin_m, gw, off, G, ncall, bases, maxg, np_g)
            in_maps.append({
                "h_src": h_src, "idx": idx_dev, "s_sel": s_arr,
                "wmat": wmat, "bcol": bcol,
            })
    elif mode == "flat":
        # flat gather streams + host-streamed S (no on-device S build)
        maxg = int(os.environ.get("GCN_MAXG", "8"))
        per_core_d = _dedup_wins(per_core, nwin)
        geom = _flat_geom(per_core_d, nwin, maxg)
        gw_lo, gw_hi, off_lo, off_hi, G_lo, G_hi, ncall_lo, ncall_hi, soff = geom
        nc = _build_program_v2(nwin, *geom, maxg, N_NODES)
        np_g = np.float16 if GDTYPE == "f16" else np.float32
        h_src = np.ascontiguousarray(H.astype(np_g))
        wmat = np.ascontiguousarray(W.astype(np_g))
        bcol = np.ascontiguousarray(b.astype(np.float32).reshape(D, 1))
        in_maps = []
        for wins in per_core_d:
            idx_lo, idx_hi, s_arr = _flat_arrays(
                wins, nwin, gw_lo, gw_hi, off_lo, off_hi, ncall_lo, ncall_hi,
                soff, maxg, np_g,
            )
            in_maps.append({
                "h_src": h_src, "idx_lo": idx_lo, "idx_hi": idx_hi,
                "s_sel": s_arr, "wmat": wmat, "bcol": bcol,
            })
    else:
        # k=11 is the largest call that fits one SWDGE ring (8k+1 <= 96
        # descriptors per DMA engine); k=12 hangs the exec unit.
        maxg = int(os.environ.get("GCN_MAXG", "8"))
        ch_lo = _chunks(g_lo, maxg)
        ch_hi = _chunks(g_hi, maxg)
        # per-window effective group counts (shared across cores): only
        # gather/matmul groups that contain at least one real edge on the
        # max-count core; the rest are pure padding.
        trim = os.environ.get("GCN_TRIM", "1") == "1"
        gw_lo = []
        gw_hi = []
        for wi in range(nwin):
            m_lo = max(len(wins[wi][0][0]) for wins in per_core)
            m_hi = max(len(wins[wi][1][0]) for wins in per_core)
            gw_lo.append(min(g_lo, max(1, _ceil_div(m_lo, 128))) if trim else g_lo)
            gw_hi.append(min(g_hi, max(1, _ceil_div(m_hi, 128))) if trim else g_hi)
        nc = _build_program(nwin, g_lo, g_hi, ch_lo, ch_hi, N_NODES,
                            gw_lo=gw_lo, gw_hi=gw_hi)
        in_maps = _make_in_maps(
            H, edge_src, edge_dst, edge_weight, W, b, per_core, nwin, g_lo,
            g_hi, ch_lo, ch_hi,
        )

    if os.environ.get("GCN_SIM", "0") == "1":  # CoreSim path for testing
        from concourse.bass_interp import CoreSim

        out = np.empty((N_NODES, D), np.float32)
        for c in range(N_CORES):
            sim = CoreSim(nc)
            for k2, v2 in in_maps[c].items():
                sim.tensor(k2)[:] = v2
            sim.simulate()
            out[c * NPW : (c + 1) * NPW, :] = np.array(
                sim.tensor("outT")).T[:NPW]
        return out

    trace = os.environ.get("GCN_TRACE", "0") == "1"
    kw = {}
    if trace:
        import shutil
        td = "/tmp/gcn_ntff"
        shutil.rmtree(td, ignore_errors=True)
        os.makedirs(td, exist_ok=True)
        kw["tmpdir"] = td
    # a previously crashed NEFF can leave the exec unit transiently
    # unrecoverable; recovery has been observed to take up to a few minutes,
    # so retry with escalating backoff. Additionally, the first "successful"
    # execution after a device wedge has been observed to return corrupt
    # data, so spot-check a few output rows against an exact host
    # computation and re-execute if they mismatch.
    import time as _time

    def _run_once():
        last_err = None
        for backoff in (15, 45, 90, 0):
            try:
                return bass_utils.run_bass_kernel_spmd(
                    nc, in_maps, core_ids=list(range(N_CORES)), trace=trace,
                    **kw,
                )
            except Exception as e:
                last_err = e
                if backoff:
                    _time.sleep(backoff)
        raise last_err

    def _spot_ok(out):
        rng = np.random.default_rng(12345)
        scale = 1e-9
        errs = []
        for c in range(N_CORES):
            n = int(c * NPW + rng.integers(0, NPW))
            e0, e1 = np.searchsorted(edge_dst, [n, n + 1])
            agg = (
                (edge_weight[e0:e1, None] * H[edge_src[e0:e1]]).sum(axis=0)
                if e1 > e0
                else np.zeros(D, np.float32)
            )
            want = agg @ W + b
            errs.append(float(np.abs(out[n] - want).max()))
            scale = max(scale, float(np.abs(want).max()))
        worst = max(errs)
        return bool(worst <= 5e-3 * scale), worst / scale

    out = np.empty((N_NODES, D), np.float32)
    res = None
    for attempt in range(3):
        res = _run_once()
        for c in range(N_CORES):
            out[c * NPW : (c + 1) * NPW, :] = res.results[c]["outT"].T[:NPW]
        ok, rel = _spot_ok(out)
        if ok:
            break
        print(f"kernel: spot-check failed (rel {rel:.2e}), re-executing")
    LAST_EXEC_NS = res.exec_time_ns
    global LAST_RESULTS
    LAST_RESULTS = res
    return out



# revision 37
# speedup vs baseline: 1.0355x; 1.0355x over previous
"""GCN layer (gather -> weighted scatter-sum -> dense transform) on 8 trn2 cores.

Strategy (1-D row partitioning of destination nodes):
  - Core c owns destination nodes [c*NPW, (c+1)*NPW). edge_dst is sorted, so
    each core's edges are a contiguous slice of the edge list.
  - Within a core, dst nodes are processed in windows of 128 (the PSUM
    partition size). Every window's edges are padded to a fixed number of
    128-edge groups so all 8 cores run the same program.
  - Per 128-edge group:
      * dma_gather pulls the 128 source rows H[src] (fp16) from HBM into an
        SBUF tile G [128 edges x 128 feat] (edge e=j*128+p lands on
        partition p, slot j).
      * DVE builds S [128 edges x 128 nodes] = (iota == dstrel) * w with one
        fused tensor_scalar op.
      * TensorE accumulates aggT[feat, node] += G.T @ S in PSUM.
  - dma_gather indices are int16 (< 32768), so edges are split into a "lo"
    stream (src < 32768, gathered from H[:32768]) and a "hi" stream
    (src >= 32768, gathered from H[32768:]); both accumulate into the same
    PSUM window.
  - Final transform: out.T = W.T @ aggT (+ b) with W stationary, computed in
    512-column chunks; bias is added during the PSUM->SBUF copy (per-partition
    ACT bias, since the output is transposed: partitions = out features).
    The kernel writes out.T [128, NWIN*128] per core; the host transposes and
    concatenates.
"""

import os
import numpy as np

N_CORES = 8
N_NODES = 50000
D = 128
NPW = N_NODES // N_CORES  # 6250 dst nodes per core
WIN = 128
SPLIT = 32768  # int16-addressable row limit for dma_gather

# gather dtype: "f16" (half gather traffic, rel err ~3e-4) or "f32" (exact)
GDTYPE = os.environ.get("GCN_GDTYPE", "f16")
# number of SWDGE queues to round-robin gather calls over (1-4)
NQUEUES = int(os.environ.get("GCN_NQ", "4"))

LAST_EXEC_NS = None  # set when GCN_TRACE=1
LAST_RESULTS = None


def _ceil_div(a, b):
    return -(-a // b)


def _prep(H, edge_src, edge_dst, edge_weight, n_cores=N_CORES):
    """Host-side sharding: per-core, per-window, per-stream edge lists with
    padding to common sizes. Returns per-core arrays + common geometry."""
    nwin = _ceil_div(NPW, WIN)
    # per (core, window, stream) edge index lists
    per_core = []
    max_lo = 0
    max_hi = 0
    max_all = 0
    for c in range(n_cores):
        n0, n1 = c * NPW, (c + 1) * NPW
        e0, e1 = np.searchsorted(edge_dst, [n0, n1])
        d = edge_dst[e0:e1] - n0
        s = edge_src[e0:e1]
        w = edge_weight[e0:e1]
        wins = []
        for wi in range(nwin):
            i0, i1 = np.searchsorted(d, [wi * WIN, wi * WIN + WIN])
            sw, dw, ww = s[i0:i1], d[i0:i1] - wi * WIN, w[i0:i1]
            lo = sw < SPLIT
            wins.append(
                (
                    (sw[lo], dw[lo], ww[lo]),
                    (sw[~lo] - SPLIT, dw[~lo], ww[~lo]),
                    (sw, dw, ww),
                )
            )
            max_lo = max(max_lo, int(lo.sum()))
            max_hi = max(max_hi, int((~lo).sum()))
            max_all = max(max_all, int(i1 - i0))
        per_core.append(wins)
    g_lo = max(1, _ceil_div(max_lo, 128))
    g_hi = max(1, _ceil_div(max_hi, 128))
    g_all = max(1, _ceil_div(max_all, 128))
    return per_core, nwin, g_lo, g_hi, g_all


def _chunks(g, maxg):
    """Split g groups into maxg-sized chunks (last may be smaller). Front-
    loading maxg-size chunks (rather than near-even) minimizes call count per
    window after trimming to the window's effective group count."""
    out = []
    c0 = 0
    while c0 < g:
        k = min(maxg, g - c0)
        out.append((c0, k))
        c0 += k
    return out


def _device_arrays(wins, nwin, g, stream, chunks, np_meta_dtype, regs=None):
    """Build idx (wrapped-16 per gather call; call = (window, chunk)) +
    dstrel/weight arrays for one core and one stream ('lo'=0, 'hi'=1).

    regs is unused (kept for signature compat)."""
    ne = g * 128
    idx = np.zeros((nwin, ne), np.int16)
    drel = np.zeros((nwin, ne), np_meta_dtype)
    wgt = np.zeros((nwin, ne), np_meta_dtype)
    for wi in range(nwin):
        sw, dw, ww = wins[wi][stream]
        n = len(sw)
        idx[wi, :n] = sw.astype(np.int16)
        drel[wi, :n] = dw.astype(np_meta_dtype)
        wgt[wi, :n] = ww.astype(np_meta_dtype)
    parts = []
    for wi in range(nwin):
        for (c0, k) in chunks:
            flat = idx[wi, c0 * 128 : (c0 + k) * 128]
            parts.append(flat.reshape(-1, 16).T)  # [16, k*8]
    idx_dev = np.tile(np.concatenate(parts, axis=1), (8, 1))  # [128, nwin*g*8]
    # meta: [p, w*g + j] = value of edge e=j*128+p in window w
    drel_dev = np.ascontiguousarray(
        drel.reshape(nwin, g, 128).transpose(2, 0, 1).reshape(128, nwin * g)
    )
    wgt_dev = np.ascontiguousarray(
        wgt.reshape(nwin, g, 128).transpose(2, 0, 1).reshape(128, nwin * g)
    )
    return idx_dev, drel_dev, wgt_dev


def _device_arrays_ind(wins, nwin, g, np_meta_dtype):
    """idx (int32, natural [p, w*g+j] layout) + dstrel/weight arrays for the
    combined stream (indirect_dma_start variant)."""
    ne = g * 128
    idx = np.zeros((nwin, ne), np.int32)
    drel = np.zeros((nwin, ne), np_meta_dtype)
    wgt = np.zeros((nwin, ne), np_meta_dtype)
    for wi in range(nwin):
        sw, dw, ww = wins[wi][2]
        n = len(sw)
        idx[wi, :n] = sw
        drel[wi, :n] = dw.astype(np_meta_dtype)
        wgt[wi, :n] = ww.astype(np_meta_dtype)

    def dev(a):
        return np.ascontiguousarray(
            a.reshape(nwin, g, 128).transpose(2, 0, 1).reshape(128, nwin * g)
        )

    return dev(idx), dev(drel), dev(wgt)


def _build_program_ind(nwin, g_all, ch, n_src_rows, n_cores=N_CORES):
    """Indirect_dma_start variant: int32 indices, no lo/hi split."""
    from contextlib import ExitStack

    import concourse.bass as bass
    import concourse.tile as tile
    from concourse import bacc, mybir

    f32 = mybir.dt.float32
    gdt = mybir.dt.float16 if GDTYPE == "f16" else mybir.dt.float32
    i32 = mybir.dt.int32

    nc = bacc.Bacc(
        "TRN2", target_bir_lowering=False, debug=False, num_devices=n_cores,
    )

    npad = nwin * WIN
    h_t = nc.dram_tensor("h_src", [n_src_rows, D], gdt, kind="ExternalInput")
    idx_t = nc.dram_tensor("idx_all", [128, nwin * g_all], i32, kind="ExternalInput")
    drel_t = nc.dram_tensor("drel", [128, nwin * g_all], gdt, kind="ExternalInput")
    wgt_t = nc.dram_tensor("wgt", [128, nwin * g_all], gdt, kind="ExternalInput")
    iota_t = nc.dram_tensor("iota", [128, 128], gdt, kind="ExternalInput")
    w_t = nc.dram_tensor("wmat", [D, D], gdt, kind="ExternalInput")
    b_t = nc.dram_tensor("bcol", [D, 1], f32, kind="ExternalInput")
    out_t = nc.dram_tensor("outT", [D, npad], f32, kind="ExternalOutput")

    with tile.TileContext(nc) as tc:
        with ExitStack() as ctx:
            const = ctx.enter_context(tc.tile_pool(name="const", bufs=1))
            gpool = ctx.enter_context(tc.tile_pool(name="gather", bufs=6))
            spool = ctx.enter_context(tc.tile_pool(name="sel", bufs=3))
            opool = ctx.enter_context(tc.tile_pool(name="outsb", bufs=2))
            ps_agg = ctx.enter_context(tc.tile_pool(name="ps_agg", bufs=2, space="PSUM"))
            ps_out = ctx.enter_context(tc.tile_pool(name="ps_out", bufs=2, space="PSUM"))

            idx = const.tile(list(idx_t.shape), i32)
            drel = const.tile(list(drel_t.shape), gdt)
            wgt = const.tile(list(wgt_t.shape), gdt)
            iota = const.tile([128, 128], gdt)
            wmat = const.tile([D, D], gdt)
            bcol = const.tile([D, 1], f32)
            agg_all = const.tile([128, npad], gdt, tag="agg_all")

            for sb, dr in ((idx, idx_t), (drel, drel_t), (wgt, wgt_t),
                           (iota, iota_t), (wmat, w_t), (bcol, b_t)):
                nc.sync.dma_start(sb[:], dr[:])

            for wg in range(nwin):
                gtiles = []
                for (c0, k) in ch:
                    gt = gpool.tile([128, ch[0][1], 128], gdt, tag="g")
                    nc.gpsimd.indirect_dma_start(
                        out=gt[:, :k, :],
                        out_offset=None,
                        in_=h_t[:],
                        in_offset=bass.IndirectOffsetOnAxis(
                            ap=idx[:, wg * g_all + c0 : wg * g_all + c0 + k],
                            axis=0,
                        ),
                    )
                    gtiles.append((gt, c0, k))

                sh = (128, g_all, 128)
                c0m = wg * g_all
                s = spool.tile([128, g_all, 128], gdt, tag="sel")
                nc.vector.tensor_tensor(
                    s[:], iota[:, None, :].broadcast_to(sh),
                    drel[:, c0m : c0m + g_all, None].broadcast_to(sh),
                    mybir.AluOpType.is_equal,
                )
                nc.vector.tensor_tensor(
                    s[:], s[:], wgt[:, c0m : c0m + g_all, None].broadcast_to(sh),
                    mybir.AluOpType.mult,
                )

                psum = ps_agg.tile([128, 128], f32, tag="psagg")
                k_idx = 0
                for (gt, c0, k) in gtiles:
                    for j in range(k):
                        nc.tensor.matmul(
                            psum[:], gt[:, j, :], s[:, c0 + j, :],
                            start=(k_idx == 0), stop=(k_idx == g_all - 1),
                        )
                        k_idx += 1
                nc.scalar.copy(agg_all[:, wg * WIN : (wg + 1) * WIN], psum[:])

            CH = 512
            for t0 in range(0, npad, CH):
                n = min(CH, npad - t0)
                po = ps_out.tile([128, CH], f32, tag="psout")
                nc.tensor.matmul(
                    po[:, :n], wmat[:], agg_all[:, t0 : t0 + n],
                    start=True, stop=True,
                )
                ob = opool.tile([128, CH], f32, tag="outsb")
                nc.scalar.add(ob[:, :n], po[:, :n], bcol[:])
                nc.sync.dma_start(out_t[:, t0 : t0 + n], ob[:, :n])

    nc.compile()
    return nc


def _build_program(nwin, g_lo, g_hi, ch_lo, ch_hi, n_src_rows, n_cores=N_CORES,
                   gw_lo=None, gw_hi=None):
    """Trace the (single, SPMD-shared) Bass program."""
    from contextlib import ExitStack

    import concourse.bass as bass
    import concourse.tile as tile
    from concourse import bacc, mybir

    f32 = mybir.dt.float32
    gdt = mybir.dt.float16 if GDTYPE == "f16" else mybir.dt.float32
    i16 = mybir.dt.int16

    nc = bacc.Bacc(
        "TRN2",
        target_bir_lowering=False,
        debug=False,
        num_devices=n_cores,
        num_swdge_queues=NQUEUES,
    )
    qctr = [0]

    def next_q():
        q = qctr[0] % NQUEUES
        qctr[0] += 1
        return q

    npad = nwin * WIN
    n_lo_rows = min(SPLIT, n_src_rows)
    n_hi_rows = n_src_rows - n_lo_rows

    h_t = nc.dram_tensor("h_src", [n_src_rows, D], gdt, kind="ExternalInput")
    idx_lo_t = nc.dram_tensor(
        "idx_lo", [128, nwin * g_lo * 8], i16, kind="ExternalInput",
    )
    idx_hi_t = nc.dram_tensor(
        "idx_hi", [128, nwin * g_hi * 8], i16, kind="ExternalInput",
    )
    drel_lo_t = nc.dram_tensor("drel_lo", [128, nwin * g_lo], gdt, kind="ExternalInput")
    wgt_lo_t = nc.dram_tensor("wgt_lo", [128, nwin * g_lo], gdt, kind="ExternalInput")
    drel_hi_t = nc.dram_tensor("drel_hi", [128, nwin * g_hi], gdt, kind="ExternalInput")
    wgt_hi_t = nc.dram_tensor("wgt_hi", [128, nwin * g_hi], gdt, kind="ExternalInput")
    iota_t = nc.dram_tensor("iota", [128, 128], gdt, kind="ExternalInput")
    w_t = nc.dram_tensor("wmat", [D, D], gdt, kind="ExternalInput")
    b_t = nc.dram_tensor("bcol", [D, 1], f32, kind="ExternalInput")
    out_t = nc.dram_tensor("outT", [D, npad], f32, kind="ExternalOutput")

    with tile.TileContext(nc) as tc:
        with ExitStack() as ctx:
            const = ctx.enter_context(tc.tile_pool(name="const", bufs=1))
            gpool = ctx.enter_context(tc.tile_pool(name="gather", bufs=6))
            spool = ctx.enter_context(tc.tile_pool(name="sel", bufs=3))
            opool = ctx.enter_context(tc.tile_pool(name="outsb", bufs=2))
            ps_agg = ctx.enter_context(
                tc.tile_pool(name="ps_agg", bufs=2, space="PSUM")
            )
            ps_out = ctx.enter_context(
                tc.tile_pool(name="ps_out", bufs=2, space="PSUM")
            )

            # resident constants / metadata
            idx_lo = const.tile(list(idx_lo_t.shape), i16)
            idx_hi = const.tile(list(idx_hi_t.shape), i16)
            drel_lo = const.tile(list(drel_lo_t.shape), gdt)
            wgt_lo = const.tile(list(wgt_lo_t.shape), gdt)
            drel_hi = const.tile(list(drel_hi_t.shape), gdt)
            wgt_hi = const.tile(list(wgt_hi_t.shape), gdt)
            iota = const.tile([128, 128], gdt)
            wmat = const.tile([D, D], gdt)
            bcol = const.tile([D, 1], f32)
            agg_all = const.tile([128, npad], gdt, tag="agg_all")

            for sb, dr in (
                (idx_lo, idx_lo_t), (idx_hi, idx_hi_t),
                (drel_lo, drel_lo_t), (wgt_lo, wgt_lo_t),
                (drel_hi, drel_hi_t), (wgt_hi, wgt_hi_t),
                (iota, iota_t), (wmat, w_t), (bcol, b_t),
            ):
                nc.sync.dma_start(sb[:], dr[:])

            h_lo = h_t[0:n_lo_rows, :]
            h_hi = h_t[n_lo_rows:n_src_rows, :] if n_hi_rows > 0 else None
            use_hi = h_hi is not None

            for wg in range(nwin):
                # gather this window's edges: one SWDGE call per chunk.
                # A call of k*128 idxs needs 8k+1 SWDGE ring entries; calls
                # with 97 entries (k=12) crash the exec unit on HW, k<=8 is
                # proven safe.
                # effective groups this window (shared across cores): groups
                # beyond the max valid count are pure padding -> not gathered,
                # not matmul'd. Every issued call is fully valid, so no tile
                # region is ever read without having been written.
                gwl = gw_lo[wg] if gw_lo else g_lo
                gwh = gw_hi[wg] if gw_hi else g_hi
                gtiles_lo = []
                for ci, (c0, k) in enumerate(ch_lo):
                    ke = min(max(gwl - c0, 0), k)
                    if ke == 0:
                        continue
                    gt = gpool.tile([128, k, 128], gdt, tag=f"glo{ci}")
                    col = (wg * g_lo + c0) * 8
                    nc.gpsimd.dma_gather(
                        gt[:, :ke, :], h_lo, idx_lo[:, col : col + ke * 8],
                        num_idxs=ke * 128, num_idxs_reg=ke * 128, elem_size=D,
                        queue_num=next_q(),
                    )
                    gtiles_lo.append((gt, c0, ke))
                gtiles_hi = []
                if use_hi:
                    for ci, (c0, k) in enumerate(ch_hi):
                        ke = min(max(gwh - c0, 0), k)
                        if ke == 0:
                            continue
                        gt = gpool.tile([128, k, 128], gdt, tag=f"ghi{ci}")
                        col = (wg * g_hi + c0) * 8
                        nc.gpsimd.dma_gather(
                            gt[:, :ke, :], h_hi, idx_hi[:, col : col + ke * 8],
                            num_idxs=ke * 128, num_idxs_reg=ke * 128, elem_size=D,
                            queue_num=next_q(),
                        )
                        gtiles_hi.append((gt, c0, ke))

                # S for the window's effective groups in 2 DVE ops per stream:
                # S[p, j, n] = (n == drel[p, j]) * w[p, j] via step-0
                # broadcast APs on both operands.
                def build_s(meta_d, meta_w, g, gw, tag):
                    s = spool.tile([128, g, 128], gdt, tag=tag)
                    sh = (128, gw, 128)
                    c0m = wg * g
                    nc.vector.tensor_tensor(
                        s[:, :gw, :], iota[:, None, :].broadcast_to(sh),
                        meta_d[:, c0m : c0m + gw, None].broadcast_to(sh),
                        mybir.AluOpType.is_equal,
                    )
                    nc.vector.tensor_tensor(
                        s[:, :gw, :], s[:, :gw, :],
                        meta_w[:, c0m : c0m + gw, None].broadcast_to(sh),
                        mybir.AluOpType.mult,
                    )
                    return s

                s_lo = build_s(drel_lo, wgt_lo, g_lo, gwl, "slo")
                s_hi = build_s(drel_hi, wgt_hi, g_hi, gwh, "shi") if use_hi else None

                psum = ps_agg.tile([128, 128], f32, tag="psagg")
                n_groups = sum(k for _, _, k in gtiles_lo)
                n_groups += sum(k for _, _, k in gtiles_hi)
                k_idx = 0
                for (gt, c0, k), s_all in (
                    [(t, s_lo) for t in gtiles_lo]
                    + [(t, s_hi) for t in gtiles_hi]
                ):
                    for j in range(k):
                        nc.tensor.matmul(
                            psum[:], gt[:, j, :], s_all[:, c0 + j, :],
                            start=(k_idx == 0), stop=(k_idx == n_groups - 1),
                        )
                        k_idx += 1
                # aggT window -> SBUF (cast to gather dtype)
                nc.scalar.copy(agg_all[:, wg * WIN : (wg + 1) * WIN], psum[:])

            # out.T = W.T @ aggT + b, in 512-column chunks
            CH = 512
            for t0 in range(0, npad, CH):
                n = min(CH, npad - t0)
                po = ps_out.tile([128, CH], f32, tag="psout")
                nc.tensor.matmul(
                    po[:, :n], wmat[:], agg_all[:, t0 : t0 + n],
                    start=True, stop=True,
                )
                ob = opool.tile([128, CH], f32, tag="outsb")
                nc.scalar.add(ob[:, :n], po[:, :n], bcol[:])
                nc.sync.dma_start(out_t[:, t0 : t0 + n], ob[:, :n])

    nc.compile()
    return nc


def _prep_merged(H, edge_src, edge_dst, edge_weight, n_cores=N_CORES):
    """One gather stream per core: per dst-window, unique srcs sorted
    ascending (even windows) / descending (odd windows) so that consecutive
    windows' slot values are continuous and any 8-group call spans a small
    src range (addressable with int16 idx off a per-call base row)."""
    nwin = _ceil_div(NPW, WIN)
    per_core = []
    for c in range(n_cores):
        n0, n1 = c * NPW, (c + 1) * NPW
        e0, e1 = np.searchsorted(edge_dst, [n0, n1])
        d = edge_dst[e0:e1] - n0
        s = edge_src[e0:e1]
        w = edge_weight[e0:e1]
        wins = []
        for wi in range(nwin):
            i0, i1 = np.searchsorted(d, [wi * WIN, wi * WIN + WIN])
            sw, dw, ww = s[i0:i1], d[i0:i1] - wi * WIN, w[i0:i1]
            if len(sw):
                u, inv = np.unique(sw, return_inverse=True)
                if wi % 2 == 1:
                    u = u[::-1].copy()
                    inv = len(u) - 1 - inv
            else:
                u, inv = sw, np.zeros(0, np.int64)
            wins.append((u, dw, ww, inv))
        per_core.append(wins)
    return per_core, nwin


def _merged_geom(per_core, nwin, maxg):
    """Group counts/offsets + per-call base rows (shared across cores)."""
    gw = []
    for wi in range(nwin):
        m = max(len(wins[wi][0]) for wins in per_core)
        gw.append(max(1, _ceil_div(m, 128)))
    off = [0]
    for wi in range(nwin):
        off.append(off[-1] + gw[wi])
    G = off[-1]
    ncall = _ceil_div(G, maxg)
    # per-call min/max real src value across cores
    bases = []
    for c in range(ncall):
        g0, g1 = c * maxg, min((c + 1) * maxg, G)
        lo_v, hi_v = N_NODES, -1
        for wins in per_core:
            for wi in range(nwin):
                if off[wi + 1] <= g0 or off[wi] >= g1:
                    continue
                u = wins[wi][0]
                if not len(u):
                    continue
                a = max(0, (g0 - off[wi]) * 128)
                b = min(len(u), (g1 - off[wi]) * 128)
                if a < b:
                    seg = u[a:b]
                    lo_v = min(lo_v, int(seg.min()))
                    hi_v = max(hi_v, int(seg.max()))
        assert hi_v >= 0, f"call {c} has no real edges"
        assert hi_v - lo_v < 32768, (c, lo_v, hi_v)
        bases.append(lo_v)
    return gw, off, G, ncall, bases


def _merged_arrays(wins, nwin, gw, off, G, ncall, bases, maxg, np_g):
    """Per-core merged idx stream (per-call base-relative, wrapped-16) and
    w-baked S stream."""
    idx = np.zeros(ncall * maxg * 128, np.int32)
    for wi in range(nwin):
        u = wins[wi][0]
        idx[off[wi] * 128 : off[wi] * 128 + len(u)] = u
    # pad slots hold 0 (absolute); make them base-relative 0 instead
    parts = []
    for c in range(ncall):
        fl = idx[c * maxg * 128 : (c + 1) * maxg * 128] - bases[c]
        real = np.zeros(maxg * 128, bool)
        g0, g1 = c * maxg, min((c + 1) * maxg, G)
        for wi in range(nwin):
            a = max(off[wi], g0)
            b = min(off[wi + 1], g1)
            if a < b:
                u_n = len(wins[wi][0])
                s0 = (a - off[wi]) * 128
                s1 = min(u_n, (b - off[wi]) * 128)
                if s1 > s0:
                    lo = a * 128 - c * maxg * 128
                    real[lo : lo + (s1 - s0)] = True
        fl[~real] = 0
        assert fl.min() >= 0 and fl.max() < 32768, (c, fl.min(), fl.max())
        parts.append(fl.astype(np.int16).reshape(-1, 16).T)
    idx_dev = np.tile(np.concatenate(parts, axis=1), (8, 1))

    s_arr = np.zeros((128, G * 128), np.float32)
    for wi in range(nwin):
        _, dw, ww, inv = wins[wi]
        if len(dw) == 0:
            continue
        np.add.at(
            s_arr,
            (inv % 128, (off[wi] + inv // 128) * 128 + dw),
            ww.astype(np.float32),
        )
    return idx_dev, s_arr.astype(np_g)


def _straddle_geom(per_core, nwin, maxg):
    """Slot-space geometry with NO per-window group padding: window w owns
    flat slots [M[w], M[w]+m[w]); groups of 128 slots may straddle window
    boundaries (such a group feeds both windows' matmuls with different,
    host-zeroed S blocks)."""
    m = [max(len(wins[wi][0]) for wins in per_core) for wi in range(nwin)]
    M = [0]
    for wi in range(nwin):
        M.append(M[-1] + m[wi])
    G = _ceil_div(M[-1], 128)
    ncall = _ceil_div(G, maxg)
    wg0 = [M[wi] // 128 for wi in range(nwin)]
    wg1 = [_ceil_div(M[wi + 1], 128) for wi in range(nwin)]
    soff = [0]
    for wi in range(nwin):
        soff.append(soff[-1] + wg1[wi] - wg0[wi])
    # per-call base rows (shared across cores)
    bases = []
    for c in range(ncall):
        s0, s1 = c * maxg * 128, min((c + 1) * maxg * 128, G * 128)
        lo_v, hi_v = N_NODES, -1
        for wins in per_core:
            for wi in range(nwin):
                if M[wi + 1] <= s0 or M[wi] >= s1:
                    continue
                u = wins[wi][0]
                a = max(0, s0 - M[wi])
                b = min(len(u), s1 - M[wi])
                if a < b:
                    seg = u[a:b]
                    lo_v = min(lo_v, int(seg.min()))
                    hi_v = max(hi_v, int(seg.max()))
        assert hi_v >= 0, f"call {c} has no real edges"
        assert hi_v - lo_v < 32768, (c, lo_v, hi_v)
        bases.append(lo_v)
    return m, M, G, ncall, wg0, wg1, soff, bases


def _straddle_arrays(wins, nwin, m, M, G, ncall, wg0, wg1, soff, bases, maxg,
                     np_g):
    """Per-core idx stream (per-call base-relative) + S stream where each
    window's blocks cover its overlapping groups (zero outside the window)."""
    idx = np.zeros(ncall * maxg * 128, np.int32)
    for wi in range(nwin):
        u = wins[wi][0]
        idx[M[wi] : M[wi] + len(u)] = u
    parts = []
    for c in range(ncall):
        s0, s1 = c * maxg * 128, (c + 1) * maxg * 128
        fl = idx[s0:s1] - bases[c]
        real = np.zeros(maxg * 128, bool)
        for wi in range(nwin):
            if M[wi + 1] <= s0 or M[wi] >= s1:
                continue
            a = max(M[wi], s0)
            b = min(M[wi] + len(wins[wi][0]), s1)
            if a < b:
                real[a - s0 : b - s0] = True
        fl[~real] = 0
        assert fl.min() >= 0 and fl.max() < 32768, (c, fl.min(), fl.max())
        parts.append(fl.astype(np.int16).reshape(-1, 16).T)
    idx_dev = np.tile(np.concatenate(parts, axis=1), (8, 1))

    s_arr = np.zeros((128, soff[-1] * 128), np.float32)
    for wi in range(nwin):
        _, dw, ww, inv = wins[wi]
        if len(dw) == 0:
            continue
        slot = M[wi] + inv
        blk = slot // 128 - wg0[wi]
        np.add.at(
            s_arr,
            (slot % 128, (soff[wi] + blk) * 128 + dw),
            ww.astype(np.float32),
        )
    return idx_dev, s_arr.astype(np_g)


def _dedup_wins(per_core, nwin):
    """Per (core, window, stream): unique sources + per-edge slot mapping.
    Edges sharing a src within a window share one gather slot; their weights
    land in different (or the same, summed) columns of that slot's S row."""
    out = []
    for wins in per_core:
        w2 = []
        for wi in range(nwin):
            entry = []
            for stream in (0, 1):
                sw, dw, ww = wins[wi][stream]
                if len(sw):
                    u, inv = np.unique(sw, return_inverse=True)
                else:
                    u, inv = sw, np.zeros(0, np.int64)
                entry.append((u, dw, ww, inv))
            w2.append(entry)
        out.append(w2)
    return out


def _flat_geom(per_core, nwin, maxg):
    """Flat-stream geometry: per-window effective (trimmed) group counts,
    flat-stream group offsets, call counts, and S-stream group offsets."""
    gw_lo, gw_hi = [], []
    for wi in range(nwin):
        m_lo = max(len(wins[wi][0][0]) for wins in per_core)
        m_hi = max(len(wins[wi][1][0]) for wins in per_core)
        gw_lo.append(max(1, _ceil_div(m_lo, 128)))
        gw_hi.append(max(1, _ceil_div(m_hi, 128)))
    off_lo = [0]
    off_hi = [0]
    soff = [0]
    for wi in range(nwin):
        off_lo.append(off_lo[-1] + gw_lo[wi])
        off_hi.append(off_hi[-1] + gw_hi[wi])
        soff.append(soff[-1] + gw_lo[wi] + gw_hi[wi])
    G_lo, G_hi = off_lo[-1], off_hi[-1]
    ncall_lo = _ceil_div(G_lo, maxg)
    ncall_hi = _ceil_div(G_hi, maxg)
    return gw_lo, gw_hi, off_lo, off_hi, G_lo, G_hi, ncall_lo, ncall_hi, soff


def _flat_arrays(wins, nwin, gw_lo, gw_hi, off_lo, off_hi, ncall_lo, ncall_hi,
                 soff, maxg, np_g):
    """Per-core flat idx streams (wrapped-16 per call) + w-baked S stream."""

    def idx_stream(stream_id, gw, off, ncall):
        idx = np.zeros(ncall * maxg * 128, np.int16)
        for wi in range(nwin):
            sw = wins[wi][stream_id][0]
            idx[off[wi] * 128 : off[wi] * 128 + len(sw)] = sw.astype(np.int16)
        parts = [
            idx[c * maxg * 128 : (c + 1) * maxg * 128].reshape(-1, 16).T
            for c in range(ncall)
        ]
        return np.tile(np.concatenate(parts, axis=1), (8, 1))

    idx_lo = idx_stream(0, gw_lo, off_lo, ncall_lo)
    idx_hi = idx_stream(1, gw_hi, off_hi, ncall_hi)

    # S with w baked in; edges sharing a gather slot (dedup) accumulate into
    # that slot's row (fp32 accumulate, cast once at the end).
    s_arr = np.zeros((128, soff[-1] * 128), np.float32)
    for wi in range(nwin):
        for stream_id, gbase in ((0, soff[wi]), (1, soff[wi] + gw_lo[wi])):
            _, dw, ww, inv = wins[wi][stream_id]
            if len(dw) == 0:
                continue
            slot = inv
            np.add.at(
                s_arr,
                (slot % 128, (gbase + slot // 128) * 128 + dw),
                ww.astype(np.float32),
            )
    return idx_lo, idx_hi, s_arr.astype(np_g)


def _build_program_v3(nwin, gw, off, G, ncall, bases, maxg, n_src_rows,
                      n_cores=N_CORES):
    """Merged single-table stream: per-call base row (compile-time), flat
    8-group calls, host-streamed S, interleaved final transform."""
    from contextlib import ExitStack

    import concourse.tile as tile
    from concourse import bacc, mybir

    f32 = mybir.dt.float32
    gdt = mybir.dt.float16 if GDTYPE == "f16" else mybir.dt.float32
    i16 = mybir.dt.int16

    nc = bacc.Bacc(
        "TRN2", target_bir_lowering=False, debug=False, num_devices=n_cores,
        num_swdge_queues=NQUEUES,
    )
    qctr = [0]

    def next_q():
        q = qctr[0] % NQUEUES
        qctr[0] += 1
        return q

    npad = nwin * WIN
    gmax = max(gw)

    h_t = nc.dram_tensor("h_src", [n_src_rows, D], gdt, kind="ExternalInput")
    idx_t = nc.dram_tensor("idx", [128, ncall * maxg * 8], i16, kind="ExternalInput")
    s_t = nc.dram_tensor("s_sel", [128, G * 128], gdt, kind="ExternalInput")
    w_t = nc.dram_tensor("wmat", [D, D], gdt, kind="ExternalInput")
    b_t = nc.dram_tensor("bcol", [D, 1], f32, kind="ExternalInput")
    out_t = nc.dram_tensor("outT", [D, npad], f32, kind="ExternalOutput")

    with tile.TileContext(nc) as tc:
        with ExitStack() as ctx:
            const = ctx.enter_context(tc.tile_pool(name="const", bufs=1))
            gpool = ctx.enter_context(tc.tile_pool(name="gather", bufs=14))
            spool = ctx.enter_context(tc.tile_pool(name="sel", bufs=4))
            opool = ctx.enter_context(tc.tile_pool(name="outsb", bufs=2))
            ps_agg = ctx.enter_context(tc.tile_pool(name="ps_agg", bufs=2, space="PSUM"))
            ps_out = ctx.enter_context(tc.tile_pool(name="ps_out", bufs=2, space="PSUM"))

            idx = const.tile(list(idx_t.shape), i16)
            wmat = const.tile([D, D], gdt)
            bcol = const.tile([D, 1], f32)
            agg_all = const.tile([128, npad], gdt, tag="agg_all")

            nc.sync.dma_start(wmat[:], w_t[:])
            nc.sync.dma_start(bcol[:], b_t[:])
            ncol = idx_t.shape[1]
            step = _ceil_div(ncol, 6)
            for a in range(0, ncol, step):
                bnd = min(a + step, ncol)
                nc.sync.dma_start(idx[:, a:bnd], idx_t[:, a:bnd])

            tiles = {}
            issued = [0]
            nreg = {}  # ke -> hoisted num_idxs register (kills per-call MOVE)

            def nref(ke):
                if ke not in nreg:
                    nreg[ke] = nc.gpsimd.to_reg(ke * 128)
                return nreg[ke]

            def issue_through(need_groups):
                while issued[0] * maxg < need_groups:
                    c = issued[0]
                    ke = min(maxg, G - c * maxg)
                    base = bases[c]
                    span = min(32768, n_src_rows - base)
                    gt = gpool.tile([128, maxg, 128], gdt, tag="g")
                    nc.gpsimd.dma_gather(
                        gt[:, :ke, :], h_t[base : base + span, :],
                        idx[:, c * maxg * 8 : c * maxg * 8 + ke * 8],
                        num_idxs=ke * 128, num_idxs_reg=nref(ke),
                        elem_size=D, queue_num=next_q(),
                    )
                    tiles[c] = gt
                    issued[0] += 1

            for wg in range(nwin):
                issue_through(off[wg] + gw[wg])

                gtot = gw[wg]
                s2 = spool.tile([128, gmax * 128], gdt, tag="s")
                nc.sync.dma_start(
                    s2[:, : gtot * 128],
                    s_t[:, off[wg] * 128 : (off[wg] + gtot) * 128],
                )

                psum = ps_agg.tile([128, 128], f32, tag="psagg")
                for j in range(gtot):
                    fg = off[wg] + j
                    nc.tensor.matmul(
                        psum[:], tiles[fg // maxg][:, fg % maxg, :],
                        s2[:, j * 128 : (j + 1) * 128],
                        start=(j == 0), stop=(j == gtot - 1),
                    )
                nc.scalar.copy(agg_all[:, wg * WIN : (wg + 1) * WIN], psum[:])

                CH = 512
                done = (wg + 1) * WIN
                emit = []
                if done % CH == 0:
                    emit.append(done - CH)
                if wg == nwin - 1 and npad % CH != 0:
                    emit.append(npad - npad % CH)
                for t0 in emit:
                    n = min(CH, npad - t0)
                    po = ps_out.tile([128, CH], f32, tag="psout")
                    nc.tensor.matmul(
                        po[:, :n], wmat[:], agg_all[:, t0 : t0 + n],
                        start=True, stop=True,
                    )
                    ob = opool.tile([128, CH], f32, tag="outsb")
                    nc.scalar.add(ob[:, :n], po[:, :n], bcol[:])
                    # out DMA on the activation engine's HWDGE queue so it
                    # never delays the sync queue's S-stream loads
                    nc.scalar.dma_start(out_t[:, t0 : t0 + n], ob[:, :n])

    nc.compile()
    return nc


def _build_program_v4(nwin, m, M, G, ncall, wg0, wg1, soff, bases, maxg,
                      n_src_rows, n_cores=N_CORES):
    """Straddle variant: zero group padding; a boundary-straddling group
    feeds both adjacent windows' matmuls with different S blocks."""
    from contextlib import ExitStack

    import concourse.tile as tile
    from concourse import bacc, mybir

    f32 = mybir.dt.float32
    gdt = mybir.dt.float16 if GDTYPE == "f16" else mybir.dt.float32
    i16 = mybir.dt.int16

    nc = bacc.Bacc(
        "TRN2", target_bir_lowering=False, debug=False, num_devices=n_cores,
        num_swdge_queues=NQUEUES,
    )
    qctr = [0]

    def next_q():
        q = qctr[0] % NQUEUES
        qctr[0] += 1
        return q

    npad = nwin * WIN
    gmax = max(wg1[i] - wg0[i] for i in range(nwin))

    h_t = nc.dram_tensor("h_src", [n_src_rows, D], gdt, kind="ExternalInput")
    idx_t = nc.dram_tensor("idx", [128, ncall * maxg * 8], i16, kind="ExternalInput")
    s_t = nc.dram_tensor("s_sel", [128, soff[-1] * 128], gdt, kind="ExternalInput")
    w_t = nc.dram_tensor("wmat", [D, D], gdt, kind="ExternalInput")
    b_t = nc.dram_tensor("bcol", [D, 1], f32, kind="ExternalInput")
    out_t = nc.dram_tensor("outT", [D, npad], f32, kind="ExternalOutput")

    with tile.TileContext(nc) as tc:
        with ExitStack() as ctx:
            const = ctx.enter_context(tc.tile_pool(name="const", bufs=1))
            gpool = ctx.enter_context(tc.tile_pool(name="gather", bufs=14))
            spool = ctx.enter_context(tc.tile_pool(name="sel", bufs=4))
            opool = ctx.enter_context(tc.tile_pool(name="outsb", bufs=2))
            ps_agg = ctx.enter_context(tc.tile_pool(name="ps_agg", bufs=2, space="PSUM"))
            ps_out = ctx.enter_context(tc.tile_pool(name="ps_out", bufs=2, space="PSUM"))

            idx = const.tile(list(idx_t.shape), i16)
            wmat = const.tile([D, D], gdt)
            bcol = const.tile([D, 1], f32)
            agg_all = const.tile([128, npad], gdt, tag="agg_all")

            nc.sync.dma_start(wmat[:], w_t[:])
            nc.sync.dma_start(bcol[:], b_t[:])
            ncol = idx_t.shape[1]
            step = _ceil_div(ncol, 6)
            for a in range(0, ncol, step):
                bnd = min(a + step, ncol)
                nc.sync.dma_start(idx[:, a:bnd], idx_t[:, a:bnd])

            tiles = {}
            issued = [0]
            nreg = {}

            def nref(ke):
                if ke not in nreg:
                    nreg[ke] = nc.gpsimd.to_reg(ke * 128)
                return nreg[ke]

            def issue_through(need_groups):
                while issued[0] * maxg < need_groups:
                    c = issued[0]
                    ke = min(maxg, G - c * maxg)
                    base = bases[c]
                    span = min(32768, n_src_rows - base)
                    gt = gpool.tile([128, maxg, 128], gdt, tag="g")
                    nc.gpsimd.dma_gather(
                        gt[:, :ke, :], h_t[base : base + span, :],
                        idx[:, c * maxg * 8 : c * maxg * 8 + ke * 8],
                        num_idxs=ke * 128, num_idxs_reg=nref(ke),
                        elem_size=D, queue_num=next_q(),
                    )
                    tiles[c] = gt
                    issued[0] += 1

            for wg in range(nwin):
                issue_through(wg1[wg])

                nblk = wg1[wg] - wg0[wg]
                s2 = spool.tile([128, gmax * 128], gdt, tag="s")
                nc.sync.dma_start(
                    s2[:, : nblk * 128],
                    s_t[:, soff[wg] * 128 : (soff[wg] + nblk) * 128],
                )

                psum = ps_agg.tile([128, 128], f32, tag="psagg")
                for bi in range(nblk):
                    g = wg0[wg] + bi
                    nc.tensor.matmul(
                        psum[:], tiles[g // maxg][:, g % maxg, :],
                        s2[:, bi * 128 : (bi + 1) * 128],
                        start=(bi == 0), stop=(bi == nblk - 1),
                    )
                nc.scalar.copy(agg_all[:, wg * WIN : (wg + 1) * WIN], psum[:])

                CH = 512
                done = (wg + 1) * WIN
                emit = []
                if done % CH == 0:
                    emit.append(done - CH)
                if wg == nwin - 1 and npad % CH != 0:
                    emit.append(npad - npad % CH)
                for t0 in emit:
                    n = min(CH, npad - t0)
                    po = ps_out.tile([128, CH], f32, tag="psout")
                    nc.tensor.matmul(
                        po[:, :n], wmat[:], agg_all[:, t0 : t0 + n],
                        start=True, stop=True,
                    )
                    ob = opool.tile([128, CH], f32, tag="outsb")
                    nc.scalar.add(ob[:, :n], po[:, :n], bcol[:])
                    nc.scalar.dma_start(out_t[:, t0 : t0 + n], ob[:, :n])

    nc.compile()
    return nc


def _build_program_v2(nwin, gw_lo, gw_hi, off_lo, off_hi, G_lo, G_hi,
                      ncall_lo, ncall_hi, soff, maxg, n_src_rows,
                      n_cores=N_CORES):
    """Flat gather streams (calls span window boundaries -> minimal call
    count) + host-precomputed S stream (w baked in) -> no DVE work."""
    from contextlib import ExitStack

    import concourse.tile as tile
    from concourse import bacc, mybir

    f32 = mybir.dt.float32
    gdt = mybir.dt.float16 if GDTYPE == "f16" else mybir.dt.float32
    i16 = mybir.dt.int16

    nc = bacc.Bacc(
        "TRN2", target_bir_lowering=False, debug=False, num_devices=n_cores,
        num_swdge_queues=NQUEUES,
    )
    qctr = [0]

    def next_q():
        q = qctr[0] % NQUEUES
        qctr[0] += 1
        return q

    npad = nwin * WIN
    gmax = max(gw_lo[i] + gw_hi[i] for i in range(nwin))
    n_lo_rows = min(SPLIT, n_src_rows)

    h_t = nc.dram_tensor("h_src", [n_src_rows, D], gdt, kind="ExternalInput")
    idx_lo_t = nc.dram_tensor(
        "idx_lo", [128, ncall_lo * maxg * 8], i16, kind="ExternalInput")
    idx_hi_t = nc.dram_tensor(
        "idx_hi", [128, ncall_hi * maxg * 8], i16, kind="ExternalInput")
    s_t = nc.dram_tensor("s_sel", [128, soff[-1] * 128], gdt, kind="ExternalInput")
    w_t = nc.dram_tensor("wmat", [D, D], gdt, kind="ExternalInput")
    b_t = nc.dram_tensor("bcol", [D, 1], f32, kind="ExternalInput")
    out_t = nc.dram_tensor("outT", [D, npad], f32, kind="ExternalOutput")

    with tile.TileContext(nc) as tc:
        with ExitStack() as ctx:
            const = ctx.enter_context(tc.tile_pool(name="const", bufs=1))
            gpool = ctx.enter_context(tc.tile_pool(name="gather", bufs=10))
            spool = ctx.enter_context(tc.tile_pool(name="sel", bufs=3))
            opool = ctx.enter_context(tc.tile_pool(name="outsb", bufs=2))
            ps_agg = ctx.enter_context(tc.tile_pool(name="ps_agg", bufs=2, space="PSUM"))
            ps_out = ctx.enter_context(tc.tile_pool(name="ps_out", bufs=2, space="PSUM"))

            idx_lo = const.tile(list(idx_lo_t.shape), i16)
            idx_hi = const.tile(list(idx_hi_t.shape), i16)
            wmat = const.tile([D, D], gdt)
            bcol = const.tile([D, 1], f32)
            agg_all = const.tile([128, npad], gdt, tag="agg_all")

            # load idx streams in column chunks so the first gather only
            # depends on the first chunk, not the whole stream
            nc.sync.dma_start(wmat[:], w_t[:])
            nc.sync.dma_start(bcol[:], b_t[:])
            # interleave lo/hi chunks so window 0's hi gathers aren't stuck
            # behind the whole lo stream
            def chunk_ranges(dr, nch=4):
                ncol = dr.shape[1]
                step = _ceil_div(ncol, nch)
                return [(a, min(a + step, ncol)) for a in range(0, ncol, step)]

            for (la, lb), (ha, hb) in zip(chunk_ranges(idx_lo_t),
                                          chunk_ranges(idx_hi_t)):
                nc.sync.dma_start(idx_lo[:, la:lb], idx_lo_t[:, la:lb])
                nc.sync.dma_start(idx_hi[:, ha:hb], idx_hi_t[:, ha:hb])

            h_lo = h_t[0:n_lo_rows, :]
            h_hi = h_t[n_lo_rows:n_src_rows, :]

            lo_tiles = {}
            hi_tiles = {}
            issued = [0, 0]

            def issue_through(which, need_groups):
                G, ncall, tiles, h_ap, idx, tag = (
                    (G_lo, ncall_lo, lo_tiles, h_lo, idx_lo, "glo")
                    if which == 0
                    else (G_hi, ncall_hi, hi_tiles, h_hi, idx_hi, "ghi")
                )
                while issued[which] * maxg < need_groups:
                    c = issued[which]
                    ke = min(maxg, G - c * maxg)
                    gt = gpool.tile([128, maxg, 128], gdt, tag=tag)
                    nc.gpsimd.dma_gather(
                        gt[:, :ke, :], h_ap,
                        idx[:, c * maxg * 8 : c * maxg * 8 + ke * 8],
                        num_idxs=ke * 128, num_idxs_reg=ke * 128,
                        elem_size=D, queue_num=next_q(),
                    )
                    tiles[c] = gt
                    issued[which] += 1

            for wg in range(nwin):
                issue_through(0, off_lo[wg] + gw_lo[wg])
                issue_through(1, off_hi[wg] + gw_hi[wg])

                gtot = gw_lo[wg] + gw_hi[wg]
                s2 = spool.tile([128, gmax * 128], gdt, tag="s")
                nc.sync.dma_start(
                    s2[:, : gtot * 128],
                    s_t[:, soff[wg] * 128 : (soff[wg] + gtot) * 128],
                )

                psum = ps_agg.tile([128, 128], f32, tag="psagg")
                k_idx = 0
                for j in range(gw_lo[wg]):
                    fg = off_lo[wg] + j
                    nc.tensor.matmul(
                        psum[:], lo_tiles[fg // maxg][:, fg % maxg, :],
                        s2[:, j * 128 : (j + 1) * 128],
                        start=(k_idx == 0), stop=(k_idx == gtot - 1),
                    )
                    k_idx += 1
                for j in range(gw_hi[wg]):
                    fg = off_hi[wg] + j
                    col = (gw_lo[wg] + j) * 128
                    nc.tensor.matmul(
                        psum[:], hi_tiles[fg // maxg][:, fg % maxg, :],
                        s2[:, col : col + 128],
                        start=(k_idx == 0), stop=(k_idx == gtot - 1),
                    )
                    k_idx += 1
                nc.scalar.copy(agg_all[:, wg * WIN : (wg + 1) * WIN], psum[:])

            CH = 512
            for t0 in range(0, npad, CH):
                n = min(CH, npad - t0)
                po = ps_out.tile([128, CH], f32, tag="psout")
                nc.tensor.matmul(
                    po[:, :n], wmat[:], agg_all[:, t0 : t0 + n],
                    start=True, stop=True,
                )
                ob = opool.tile([128, CH], f32, tag="outsb")
                nc.scalar.add(ob[:, :n], po[:, :n], bcol[:])
                nc.sync.dma_start(out_t[:, t0 : t0 + n], ob[:, :n])

    nc.compile()
    return nc


def _make_in_maps(H, edge_src, edge_dst, edge_weight, W, b, per_core, nwin,
                  g_lo, g_hi, ch_lo, ch_hi):
    np_g = np.float16 if GDTYPE == "f16" else np.float32
    h_src = np.ascontiguousarray(H.astype(np_g))
    iota = np.tile(np.arange(128, dtype=np_g), (128, 1))
    wmat = np.ascontiguousarray(W.astype(np_g))
    bcol = np.ascontiguousarray(b.astype(np.float32).reshape(D, 1))
    in_maps = []
    for wins in per_core:
        idx_lo, drel_lo, wgt_lo = _device_arrays(wins, nwin, g_lo, 0, ch_lo, np_g)
        idx_hi, drel_hi, wgt_hi = _device_arrays(wins, nwin, g_hi, 1, ch_hi, np_g)
        in_maps.append(
            {
                "h_src": h_src,
                "idx_lo": idx_lo, "idx_hi": idx_hi,
                "drel_lo": drel_lo, "wgt_lo": wgt_lo,
                "drel_hi": drel_hi, "wgt_hi": wgt_hi,
                "iota": iota, "wmat": wmat, "bcol": bcol,
            }
        )
    return in_maps


def kernel(H, edge_src, edge_dst, edge_weight, W, b):
    global LAST_EXEC_NS
    from concourse import bass_utils

    H = np.asarray(H, dtype=np.float32)
    edge_src = np.asarray(edge_src, dtype=np.int32)
    edge_dst = np.asarray(edge_dst, dtype=np.int32)
    edge_weight = np.asarray(edge_weight, dtype=np.float32)
    W = np.asarray(W, dtype=np.float32)
    b = np.asarray(b, dtype=np.float32)

    per_core, nwin, g_lo, g_hi, g_all = _prep(H, edge_src, edge_dst, edge_weight)
    mode = os.environ.get("GCN_GATHER", "straddle")
    if mode == "indirect":
        maxg = int(os.environ.get("GCN_MAXG", "8"))
        ch = _chunks(g_all, maxg)
        nc = _build_program_ind(nwin, g_all, ch, N_NODES)
        np_g = np.float16 if GDTYPE == "f16" else np.float32
        h_src = np.ascontiguousarray(H.astype(np_g))
        iota = np.tile(np.arange(128, dtype=np_g), (128, 1))
        wmat = np.ascontiguousarray(W.astype(np_g))
        bcol = np.ascontiguousarray(b.astype(np.float32).reshape(D, 1))
        in_maps = []
        for wins in per_core:
            idx_all, drel, wgt = _device_arrays_ind(wins, nwin, g_all, np_g)
            in_maps.append({
                "h_src": h_src, "idx_all": idx_all, "drel": drel, "wgt": wgt,
                "iota": iota, "wmat": wmat, "bcol": bcol,
            })
    elif mode == "straddle":
        # merged stream with zero group padding (groups straddle windows)
        maxg = int(os.environ.get("GCN_MAXG", "8"))
        per_core_m, nwin_m = _prep_merged(H, edge_src, edge_dst, edge_weight)
        geom = _straddle_geom(per_core_m, nwin_m, maxg)
        m_, M_, G, ncall, wg0, wg1, soff, bases = geom
        nc = _build_program_v4(nwin_m, *geom, maxg, N_NODES)
        np_g = np.float16 if GDTYPE == "f16" else np.float32
        h_src = np.ascontiguousarray(H.astype(np_g))
        wmat = np.ascontiguousarray(W.astype(np_g))
        bcol = np.ascontiguousarray(b.astype(np.float32).reshape(D, 1))
        in_maps = []
        for wins in per_core_m:
            idx_dev, s_arr = _straddle_arrays(
                wins, nwin_m, m_, M_, G, ncall, wg0, wg1, soff, bases, maxg,
                np_g)
            in_maps.append({
                "h_src": h_src, "idx": idx_dev, "s_sel": s_arr,
                "wmat": wmat, "bcol": bcol,
            })
    elif mode == "merged":
        # single-table boustrophedon stream, per-call base rows
        maxg = int(os.environ.get("GCN_MAXG", "8"))
        per_core_m, nwin_m = _prep_merged(H, edge_src, edge_dst, edge_weight)
        gw, off, G, ncall, bases = _merged_geom(per_core_m, nwin_m, maxg)
        nc = _build_program_v3(nwin_m, gw, off, G, ncall, bases, maxg, N_NODES)
        np_g = np.float16 if GDTYPE == "f16" else np.float32
        h_src = np.ascontiguousarray(H.astype(np_g))
        wmat = np.ascontiguousarray(W.astype(np_g))
        bcol = np.ascontiguousarray(b.astype(np.float32).reshape(D, 1))
        in_maps = []
        for wins in per_core_m:
            idx_dev, s_arr = _merged_arrays(
                wins, nwin_m, gw, off, G, ncall, bases, maxg, np_g)
            in_maps.append({
                "h_src": h_src, "idx": idx_dev, "s_sel": s_arr,
                "wmat": wmat, "bcol": bcol,
            })
    elif mode == "flat":
        # flat gather streams + host-streamed S (no on-device S build)
        maxg = int(os.environ.get("GCN_MAXG", "8"))
        per_core_d = _dedup_wins(per_core, nwin)
        geom = _flat_geom(per_core_d, nwin, maxg)
        gw_lo, gw_hi, off_lo, off_hi, G_lo, G_hi, ncall_lo, ncall_hi, soff = geom
        nc = _build_program_v2(nwin, *geom, maxg, N_NODES)
        np_g = np.float16 if GDTYPE == "f16" else np.float32
        h_src = np.ascontiguousarray(H.astype(np_g))
        wmat = np.ascontiguousarray(W.astype(np_g))
        bcol = np.ascontiguousarray(b.astype(np.float32).reshape(D, 1))
        in_maps = []
        for wins in per_core_d:
            idx_lo, idx_hi, s_arr = _flat_arrays(
                wins, nwin, gw_lo, gw_hi, off_lo, off_hi, ncall_lo, ncall_hi,
                soff, maxg, np_g,
            )
            in_maps.append({
                "h_src": h_src, "idx_lo": idx_lo, "idx_hi": idx_hi,
                "s_sel": s_arr, "wmat": wmat, "bcol": bcol,
            })
    else:
        # k=11 is the largest call that fits one SWDGE ring (8k+1 <= 96
        # descriptors per DMA engine); k=12 hangs the exec unit.
        maxg = int(os.environ.get("GCN_MAXG", "8"))
        ch_lo = _chunks(g_lo, maxg)
        ch_hi = _chunks(g_hi, maxg)
        # per-window effective group counts (shared across cores): only
        # gather/matmul groups that contain at least one real edge on the
        # max-count core; the rest are pure padding.
        trim = os.environ.get("GCN_TRIM", "1") == "1"
        gw_lo = []
        gw_hi = []
        for wi in range(nwin):
            m_lo = max(len(wins[wi][0][0]) for wins in per_core)
            m_hi = max(len(wins[wi][1][0]) for wins in per_core)
            gw_lo.append(min(g_lo, max(1, _ceil_div(m_lo, 128))) if trim else g_lo)
            gw_hi.append(min(g_hi, max(1, _ceil_div(m_hi, 128))) if trim else g_hi)
        nc = _build_program(nwin, g_lo, g_hi, ch_lo, ch_hi, N_NODES,
                            gw_lo=gw_lo, gw_hi=gw_hi)
        in_maps = _make_in_maps(
            H, edge_src, edge_dst, edge_weight, W, b, per_core, nwin, g_lo,
            g_hi, ch_lo, ch_hi,
        )

    if os.environ.get("GCN_SIM", "0") == "1":  # CoreSim path for testing
        from concourse.bass_interp import CoreSim

        out = np.empty((N_NODES, D), np.float32)
        for c in range(N_CORES):
            sim = CoreSim(nc)
            for k2, v2 in in_maps[c].items():
                sim.tensor(k2)[:] = v2
            sim.simulate()
            out[c * NPW : (c + 1) * NPW, :] = np.array(
                sim.tensor("outT")).T[:NPW]
        return out

    trace = os.environ.get("GCN_TRACE", "0") == "1"
    kw = {}
    if trace:
        import shutil
        td = "/tmp/gcn_ntff"
        shutil.rmtree(td, ignore_errors=True)
        os.makedirs(td, exist_ok=True)
        kw["tmpdir"] = td
    # a previously crashed NEFF can leave the exec unit transiently
    # unrecoverable; recovery has been observed to take up to a few minutes,
    # so retry with escalating backoff. Additionally, the first "successful"
    # execution after a device wedge has been observed to return corrupt
    # data, so spot-check a few output rows against an exact host
    # computation and re-execute if they mismatch.
    import time as _time

    def _run_once():
        last_err = None
        for backoff in (15, 45, 90, 0):
            try:
                return bass_utils.run_bass_kernel_spmd(
                    nc, in_maps, core_ids=list(range(N_CORES)), trace=trace,
                    **kw,
                )
            except Exception as e:
                last_err = e
                if backoff:
                    _time.sleep(backoff)
        raise last_err

    def _spot_ok(out):
        rng = np.random.default_rng(12345)
        scale = 1e-9
        errs = []
        for c in range(N_CORES):
            n = int(c * NPW + rng.integers(0, NPW))
            e0, e1 = np.searchsorted(edge_dst, [n, n + 1])
            agg = (
                (edge_weight[e0:e1, None] * H[edge_src[e0:e1]]).sum(axis=0)
                if e1 > e0
                else np.zeros(D, np.float32)
            )
            want = agg @ W + b
            errs.append(float(np.abs(out[n] - want).max()))
            scale = max(scale, float(np.abs(want).max()))
        worst = max(errs)
        return bool(worst <= 5e-3 * scale), worst / scale

    out = np.empty((N_NODES, D), np.float32)
    res = None
    for attempt in range(3):
        res = _run_once()
        for c in range(N_CORES):
            out[c * NPW : (c + 1) * NPW, :] = res.results[c]["outT"].T[:NPW]
        ok, rel = _spot_ok(out)
        if ok:
            break
        print(f"kernel: spot-check failed (rel {rel:.2e}), re-executing")
    LAST_EXEC_NS = res.exec_time_ns
    global LAST_RESULTS
    LAST_RESULTS = res
    return out

